# revision 21
# baseline (speedup 1.0000x reference)
"""GAT (graph attention) message-passing kernel for Trainium2, 8 NeuronCores.

Strategy (graph/data parallel, dst-sharded):
  - Nodes are partitioned across 8 cores by destination id (12500 each).
  - Edges are sharded by dst partition, sorted by (dst-block, src-subtable),
    and padded so every core runs an identical (SPMD) program.
  - Per step, every core projects ALL nodes (h = x @ [W | W@attn_l]) into a
    bf16 row table in its HBM ([h(256) | el(4) | pad] = 768B rows), then
    indirect-gathers h[src] rows per edge (dma_gather), builds one-hot dst
    masks on DVE, computes attention scores (er via a small maskT matmul),
    and accumulates [softmax-denominator | weighted message sum] into
    per-dst-block PSUM with mask matmuls on TensorE.
  - Block epilogue: normalize by the segment sum, head-mean, residual update.
  - Between the 2 conv steps, the updated x (transposed, bf16) is AllGathered
    across the 8 cores.
"""

import os
import math
import numpy as np
import ml_dtypes

import concourse.bass as bass
import concourse.tile as tile
import concourse.mybir as mybir
from concourse import library_config
from concourse.library_overlay import lower_extended_insts
from concourse.bass_utils import run_bass_kernel_spmd

BF16 = mybir.dt.bfloat16
F32 = mybir.dt.float32
I16 = mybir.dt.int16
AF = mybir.ActivationFunctionType
ALU = mybir.AluOpType

NEG_SLOPE = 0.2
STEP = int(os.environ.get("GAT_STEPS", "2"))
SKIP_COLL = bool(int(os.environ.get("GAT_SKIP_COLL", "0")))
SKIP_GATHER = bool(int(os.environ.get("GAT_SKIP_GATHER", "0")))
N_CORES = 8
SB = 4            # blocks per superblock (PSUM accumulators alive at once)
MAX_CALL = 16     # max 128-edge chunks per dma_gather call
GS = 8            # chunks per elementwise batch group
ST_MAX_ROWS = 25000   # subtable rows (int16 gather index limit)

_last_results = None  # BassKernelResults stash for test harness
_last_nc = None
_last_in_maps = None


def _bf(x):
    return np.asarray(x, np.float32).astype(ml_dtypes.bfloat16)


# ----------------------------------------------------------------------------
# host-side preprocessing
# ----------------------------------------------------------------------------

def _plan_and_arrays(src, dst, N):
    """Shard/sort/pad edges; build the shared chunk plan and per-core arrays."""
    Nl = N // N_CORES
    NB = (Nl + 127) // 128
    NSB = (NB + SB - 1) // SB
    NST = max(1, math.ceil(N / ST_MAX_ROWS))
    st_rows = math.ceil(N / NST)

    core = dst // Nl
    percore = []
    for p in range(N_CORES):
        sel = np.nonzero(core == p)[0]
        s = src[sel].astype(np.int64)
        d = (dst[sel] - p * Nl).astype(np.int64)
        blk = d >> 7
        st = s // st_rows
        order = np.lexsort((s, st, blk))
        percore.append((s[order], d[order], blk[order], st[order]))

    counts = np.zeros((N_CORES, NB, NST), np.int64)
    for p in range(N_CORES):
        _, _, blk, st = percore[p]
        np.add.at(counts, (p, blk, st), 1)
    nchunks = (counts.max(axis=0) + 127) // 128          # [NB, NST]

    # canonical chunk emission order
    chunk_meta = []   # (isb, st, b) per chunk
    calls = []        # (st, chunk_lo, n_chunks)
    for isb in range(NSB):
        blocks = range(isb * SB, min((isb + 1) * SB, NB))
        for st in range(NST):
            run_lo = len(chunk_meta)
            for b in blocks:
                for _ in range(int(nchunks[b, st])):
                    chunk_meta.append((isb, st, b))
            n = len(chunk_meta) - run_lo
            o = run_lo
            while n > 0:
                take = min(n, MAX_CALL)
                calls.append((st, o, take))
                o += take
                n -= take
    NCH = len(chunk_meta)

    # first/last chunk index per (isb, b) for PSUM start/stop flags
    first = {}
    last = {}
    for ci, (isb, st, b) in enumerate(chunk_meta):
        key = (isb, b)
        if key not in first:
            first[key] = ci
        last[key] = ci

    # per-core edge arrays in padded chunk order
    idx_all = np.zeros((N_CORES, NCH * 128), np.int16)
    doff_all = np.full((N_CORES, NCH * 128), 255.0, np.float32)
    for p in range(N_CORES):
        s, d, blk, st = percore[p]
        # build run boundaries of the (blk, st)-sorted edge list
        runs = {}
        i = 0
        M = len(s)
        while i < M:
            k = (blk[i], st[i])
            j = i
            while j < M and blk[j] == k[0] and st[j] == k[1]:
                j += 1
            runs[k] = (i, j)
            i = j
        cursor = {k: v[0] for k, v in runs.items()}
        for ci, (isb, t, b) in enumerate(chunk_meta):
            base = ci * 128
            k = (b, t)
            if k in runs:
                lo = cursor[k]
                hi = min(lo + 128, runs[k][1])
                n = hi - lo
                cursor[k] = hi
                if n > 0:
                    idx_all[p, base:base + n] = (s[lo:hi] - t * st_rows).astype(np.int16)
                    doff_all[p, base:base + n] = (d[lo:hi] - b * 128).astype(np.float32)
        for k, (lo, hi) in runs.items():
            assert cursor[k] == hi, "edge run not fully consumed"

    # gather-call wrapped idx layout: per call [16, n/16], concat on free axis
    idxw_cols = NCH * 8
    idx_wrapped = np.zeros((N_CORES, 16, idxw_cols), np.int16)
    col = 0
    call_cols = []
    for (t, lo, nch) in calls:
        n = nch * 128
        for p in range(N_CORES):
            seg = idx_all[p, lo * 128: lo * 128 + n]
            idx_wrapped[p, :, col:col + n // 16] = seg.reshape(-1, 16).T
        call_cols.append(col)
        col += n // 16
    assert col == idxw_cols

    # dstoff [128, NCH]: partition = edge-in-chunk
    doff = doff_all.reshape(N_CORES, NCH, 128).transpose(0, 2, 1)
    # dstrep [128, 4*NCH]: dstrep[p, 4c+j] = doff_edge(c, 32j + p%32)
    j_idx = np.arange(4)
    p_idx = np.arange(128)
    e_idx = (32 * j_idx[None, :] + (p_idx % 32)[:, None])      # [128, 4]
    dstrep = np.empty((N_CORES, 128, 4 * NCH), np.float32)
    for p in range(N_CORES):
        d3 = doff_all[p].reshape(NCH, 128)                      # [NCH, 128e]
        rep = d3[:, e_idx]                                      # [NCH, 128, 4]
        dstrep[p] = rep.transpose(1, 0, 2).reshape(128, NCH * 4)

    groups = []
    for (t, lo, nch) in calls:
        g = lo
        while g < lo + nch:
            take = min(GS, lo + nch - g)
            groups.append((t, lo, g, take))  # (st, call_lo, group_lo, size)
            g += take

    return dict(Nl=Nl, NB=NB, NSB=NSB, NST=NST, st_rows=st_rows, NCH=NCH,
                chunk_meta=chunk_meta, calls=calls, call_cols=call_cols,
                groups=groups, first=first, last=last,
                idx_wrapped=idx_wrapped, dstoff=doff, dstrep=dstrep,
                idxw_cols=idxw_cols)


# ----------------------------------------------------------------------------
# device program
# ----------------------------------------------------------------------------

def _split_multi_waits(nc):
    """walrus codegen only accepts one sync-wait per instruction; hoist any
    extra waits onto same-engine NOPs inserted right before the instruction."""
    n_id = 0
    for f in nc.m.functions:
        for blk in f.blocks:
            out = []
            for ins in blk.instructions:
                si = ins.sync_info
                if si is not None and len(si.on_wait) > 1 \
                        and ins.engine is not None:
                    waits = list(si.on_wait)
                    for w in waits[:-1]:
                        nop = mybir.InstNoOp(name=f"I-wsplit-{n_id}", ins=[],
                                             outs=[])
                        n_id += 1
                        nop.engine = ins.engine
                        nop.sync_info = mybir.SyncInfo(on_wait=[w],
                                                       on_update=[])
                        nc.inst_map[nop.name] = nop
                        out.append(nop)
                    ins.sync_info = mybir.SyncInfo(on_wait=[waits[-1]],
                                                   on_update=list(si.on_update))
                out.append(ins)
            blk.instructions = out

def _ap(base, *dims):
    """Rebuild AP with the same tensor/offset/partition dim, custom free dims."""
    return bass.AP(base.tensor, base.offset,
                   [list(base.ap[0])] + [list(d) for d in dims])


def _build(meta, N, D, H):
    Nl, NB, NSB, NST = meta["Nl"], meta["NB"], meta["NSB"], meta["NST"]
    st_rows = meta["st_rows"]
    NBP = NB * 128
    HD = H * D            # 256
    RW = HD + H           # 260 (h | el)
    TW = ((RW * 2 + 255) // 256) * 128  # 384 elems bf16 -> 768B rows

    nc = bass.Bass("TRN2", target_bir_lowering=False, debug=False,
                   enable_asserts=False, num_devices=N_CORES)

    # ---- DRAM tensors
    xT_in = nc.dram_tensor("xT_in", [D, N], BF16, kind="ExternalInput")
    xTl_in = nc.dram_tensor("xTl_in", [D, NBP], BF16, kind="ExternalInput")
    x_in = nc.dram_tensor("x_in", [128, NB, D], F32, kind="ExternalInput")
    c0_in = nc.dram_tensor("c0_in", [128, NB, D], F32, kind="ExternalInput")
    waug_in = nc.dram_tensor("waug_in", [D, RW], BF16, kind="ExternalInput")
    wr_in = nc.dram_tensor("wr_in", [D, H], BF16, kind="ExternalInput")
    iota_in = nc.dram_tensor("iota_in", [128, 128], BF16, kind="ExternalInput")
    itld_in = nc.dram_tensor("itld_in", [128, 32], BF16, kind="ExternalInput")
    ident_in = nc.dram_tensor("ident_in", [128, 128], BF16, kind="ExternalInput")
    scal_in = nc.dram_tensor("scal_in", [128, 4], F32, kind="ExternalInput")
    idx_in = nc.dram_tensor("idx_in", [128, meta["idxw_cols"]], I16,
                            kind="ExternalInput")
    doff_in = nc.dram_tensor("doff_in", [128, meta["NCH"]], BF16,
                             kind="ExternalInput")
    drep_in = nc.dram_tensor("drep_in", [128, 4 * meta["NCH"]], BF16,
                             kind="ExternalInput")

    table = nc.dram_tensor("table", [N, TW], BF16, kind="Internal")
    x_mid = nc.dram_tensor("x_mid", [128, NB, D], F32, kind="Internal")
    xT_sh = nc.dram_tensor("xT_sh", [D, NBP], BF16, kind="Internal")
    xT_ag = nc.dram_tensor("xT_ag", [D * N_CORES, NBP], BF16, kind="Internal",
                           addr_space="Shared")
    x_out = nc.dram_tensor("x_out", [Nl, D], F32, kind="ExternalOutput")

    from contextlib import ExitStack
    with tile.TileContext(nc) as tc, ExitStack() as es_:
        nc.gpsimd.load_library(library_config.mlp)
        cp = es_.enter_context(tc.tile_pool(name="consts", bufs=1))
        pools = {}
        for nm, bufs in [("xt", 4), ("rows", 3), ("mask", 2), ("rhs", 3),
                         ("sm", 3), ("tbl", 4), ("blk", 3), ("big", 2)]:
            pools[nm] = es_.enter_context(tc.tile_pool(name=nm, bufs=bufs))
        pA = es_.enter_context(tc.tile_pool(name="pacc", bufs=1, space="PSUM"))
        pB = es_.enter_context(tc.tile_pool(name="per8", bufs=2, space="PSUM"))
        pC = es_.enter_context(tc.tile_pool(name="ppj", bufs=2, space="PSUM"))

        # ---- load constants
        iota_t = cp.tile([128, 128], BF16, tag="iota")
        itld_t = cp.tile([128, 32], BF16, tag="itld")
        ident_t = cp.tile([128, 128], BF16, tag="ident")
        waug_t = cp.tile([D, RW], BF16, tag="waug")
        wr_t = cp.tile([D, H], BF16, tag="wr")
        scal_t = cp.tile([128, 4], F32, tag="scal")
        idx_t = cp.tile([128, meta["idxw_cols"]], I16, tag="idx")
        doff_t = cp.tile([128, meta["NCH"]], BF16, tag="doff")
        drep_t = cp.tile([128, 4 * meta["NCH"]], BF16, tag="drep")
        for t, s in [(iota_t, iota_in), (itld_t, itld_in), (ident_t, ident_in),
                     (waug_t, waug_in), (wr_t, wr_in), (scal_t, scal_in),
                     (idx_t, idx_in), (doff_t, doff_in), (drep_t, drep_in)]:
            nc.sync.dma_start(t[:], s.ap()[:])

        # zero-fill the table's pad columns once (the gather reads full
        # 768B rows; compute never touches the pad, but it must be finite)
        PAD = TW - RW
        zt = cp.tile([128, PAD], BF16, tag="zpad")
        nc.vector.memset(zt[:], 0)
        nrep = N // 128
        tap = table.ap()
        nc.sync.dma_start(
            bass.AP(tap.tensor, RW, [[TW, 128], [TW * 128, nrep], [1, PAD]]),
            _ap(zt[:], [0, nrep], [1, PAD]))

        tails = {NB - 1: Nl - 128 * (NB - 1)}
        nidx_regs = {}

        def nidx_reg(n):
            if n not in nidx_regs:
                nidx_regs[n] = nc.gpsimd.to_reg(n)
            return nidx_regs[n]

        for step in range(STEP):
            # ------------------------------------------------ projection
            eng_flip = 0
            for r in range(N_CORES):
                for t in range(NB):
                    o = 128 * t
                    w = min(128, Nl - o)
                    g0 = r * Nl + o
                    xt = pools["xt"].tile([D, 128], BF16, tag="projlhs")
                    if step == 0:
                        nc.sync.dma_start(xt[:, :w], xT_in.ap()[:, g0:g0 + w])
                    else:
                        nc.sync.dma_start(
                            xt[:, :w], xT_ag.ap()[D * r:D * (r + 1), o:o + w])
                    pp = pC.tile([128, RW], F32, tag="pj")
                    nc.tensor.matmul(pp[:w, :], xt[:, :w], waug_t[:],
                                     start=True, stop=True)
                    tb = pools["tbl"].tile([128, RW], BF16, tag="tbl")
                    if eng_flip % 2 == 0:
                        nc.vector.tensor_copy(tb[:w, :], pp[:w, :])
                    else:
                        nc.scalar.activation(tb[:w, :], pp[:w, :], AF.Copy)
                    eng_flip += 1
                    nc.sync.dma_start(table.ap()[g0:g0 + w, 0:RW], tb[:w, :])

            # ------------------------------------------------ gather + attn
            x_src = x_in if step == 0 else x_mid
            xt_src = xTl_in if step == 0 else xT_sh
            call_i = 0
            group_i = 0
            for isb in range(NSB):
                blocks = list(range(isb * SB, min((isb + 1) * SB, NB)))
                nb = len(blocks)
                b0 = blocks[0]
                acc = pA.tile([128, SB, 512], F32, tag="acc")
                x4 = pools["blk"].tile([128, SB, D], F32, tag="x4")
                c04 = pools["blk"].tile([128, SB, D], F32, tag="c04")
                nc.sync.dma_start(x4[:, :nb, :], x_src.ap()[:, b0:b0 + nb, :])
                nc.sync.dma_start(c04[:, :nb, :], c0_in.ap()[:, b0:b0 + nb, :])
                # x4p = (1-alpha) * x4 + c0
                x4p = pools["blk"].tile([128, SB, D], F32, tag="x4p")
                nc.vector.scalar_tensor_tensor(
                    x4p[:, :nb, :], x4[:, :nb, :], scal_t[:, 0:1],
                    c04[:, :nb, :], op0=ALU.mult, op1=ALU.add)
                er_sb = {}
                for j, b in enumerate(blocks):
                    xtb = pools["xt"].tile([D, 128], BF16, tag="erlhs")
                    nc.sync.dma_start(xtb[:], xt_src.ap()[:, 128 * b:128 * (b + 1)])
                    nc.tensor.matmul(acc[:, j, 264:264 + H], xtb[:], wr_t[:],
                                     start=True, stop=True)
                    es = pools["sm"].tile([128, H], BF16, tag="erblk%d" % j)
                    nc.scalar.activation(es[:], acc[:, j, 264:264 + H], AF.Copy)
                    er_sb[b] = es

                # walk this superblock's calls/groups/chunks
                while call_i < len(meta["calls"]):
                    st, lo, nch = meta["calls"][call_i]
                    if lo >= len(meta["chunk_meta"]) or \
                       meta["chunk_meta"][lo][0] != isb:
                        break
                    n = nch * 128
                    rows = pools["rows"].tile([128, MAX_CALL, TW], BF16,
                                              tag="rows")
                    icol = meta["call_cols"][call_i]
                    rows_ap = _ap(rows[:], [TW, nch], [1, TW])
                    tbl_ap = table.ap()[st * st_rows:
                                        min((st + 1) * st_rows, N), :]
                    if not SKIP_GATHER:
                        nc.gpsimd.dma_gather(
                            rows_ap, tbl_ap, idx_t[:, icol:icol + n // 16],
                            num_idxs=n, num_idxs_reg=nidx_reg(n), elem_size=TW,
                            single_packet=False)
                    call_i += 1

                    while group_i < len(meta["groups"]):
                        gst, glo_call, g, gs = meta["groups"][group_i]
                        if glo_call != lo:
                            break
                        group_i += 1
                        cc0 = g - lo   # chunk offset within call
                        # mask [128, gs, 128]
                        m8 = pools["mask"].tile([128, GS, 128], BF16, tag="m8")
                        nc.vector.tensor_tensor(
                            _ap(m8[:], [128, gs], [1, 128]),
                            _ap(iota_t[:], [0, gs], [1, 128]),
                            _ap(doff_t[:, g:g + gs], [1, gs], [0, 128]),
                            op=ALU.is_equal)
                        mt8 = pools["mask"].tile([128, GS, 128], BF16, tag="mt8")
                        nc.vector.tensor_tensor(
                            _ap(mt8[:], [128, gs], [1, 128]),
                            _ap(drep_t[:, 4 * g:4 * (g + gs)],
                                [4, gs], [1, 4], [0, 32]),
                            _ap(itld_t[:], [0, gs], [0, 4], [1, 32]),
                            op=ALU.is_equal)
                        mT8 = pools["mask"].tile([128, GS, 128], BF16, tag="mT8")
                        nc.vector.transpose(
                            _ap(mT8[:], [1, gs * 128]),
                            _ap(mt8[:], [1, gs * 128]))
                        er8 = pB.tile([128, GS * H], F32, tag="er8")
                        for k in range(gs):
                            ci = g + k
                            _, _, b = meta["chunk_meta"][ci]
                            nc.tensor.matmul(er8[:, H * k:H * (k + 1)],
                                             mT8[:, k, :], er_sb[b],
                                             start=True, stop=False)
                            nc.tensor.matmul(er8[:, H * k:H * (k + 1)],
                                             ident_t[:],
                                             rows[:, cc0 + k, HD:HD + H],
                                             start=False, stop=True)
                        t8 = pools["sm"].tile([128, GS * H], BF16, tag="t8")
                        nc.scalar.activation(t8[:, :gs * H], er8[:, :gs * H],
                                             AF.Copy)
                        lr8 = pools["sm"].tile([128, GS * H], BF16, tag="lr8")
                        nc.vector.scalar_tensor_tensor(
                            lr8[:, :gs * H], t8[:, :gs * H], NEG_SLOPE,
                            t8[:, :gs * H], op0=ALU.mult, op1=ALU.max)
                        rhs8 = pools["rhs"].tile([128, GS, RW], BF16, tag="rhs8")
                        nc.scalar.activation(
                            _ap(rhs8[:], [RW, gs], [1, H]),
                            _ap(lr8[:], [H, gs], [1, H]), AF.Exp)
                        nc.vector.tensor_tensor(
                            _ap(rhs8[:, :, H:RW], [RW, gs], [D, H], [1, D]),
                            _ap(rows[:, cc0:cc0 + gs, 0:HD],
                                [TW, gs], [D, H], [1, D]),
                            _ap(rhs8[:], [RW, gs], [1, H], [0, D]),
                            op=ALU.mult)
                        for k in range(gs):
                            ci = g + k
                            _, _, b = meta["chunk_meta"][ci]
                            j = b - b0
                            nc.tensor.matmul(
                                acc[:, j, 0:RW], m8[:, k, :], rhs8[:, k, :],
                                start=(meta["first"][(isb, b)] == ci),
                                stop=(meta["last"][(isb, b)] == ci),
                                skip_group_check=True)

                # ---- superblock epilogue (batched over blocks)
                smax = pools["sm"].tile([128, SB * H], F32, tag="smax")
                nc.vector.tensor_scalar(
                    _ap(smax[:], [H, nb], [1, H]),
                    _ap(acc[:], [512, nb], [1, H]),
                    1e-30, None, op0=ALU.max)
                srec = pools["sm"].tile([128, SB * H], F32, tag="srec")
                nc.vector.reciprocal(srec[:, :nb * H], smax[:, :nb * H])
                srec2 = pools["sm"].tile([128, SB * H], F32, tag="srec2")
                nc.vector.tensor_scalar(
                    srec2[:, :nb * H], srec[:, :nb * H], scal_t[:, 1:2], None,
                    op0=ALU.mult)
                onorm = pools["big"].tile([128, SB, H, D], F32, tag="onorm")
                nc.vector.tensor_tensor(
                    _ap(onorm[:], [H * D, nb], [D, H], [1, D]),
                    _ap(acc[:, :, H:RW], [512, nb], [D, H], [1, D]),
                    _ap(srec2[:], [H, nb], [1, H], [0, D]),
                    op=ALU.mult)
                red = pools["blk"].tile([128, SB, D], F32, tag="red")
                nc.vector.tensor_reduce(
                    _ap(red[:], [D, nb], [1, D]),
                    _ap(onorm[:], [H * D, nb], [1, D], [D, H]),
                    axis=mybir.AxisListType.X, op=ALU.add)
                xn = pools["blk"].tile([128, SB, D], F32, tag="xn")
                nc.vector.tensor_add(xn[:, :nb, :], x4p[:, :nb, :],
                                     red[:, :nb, :])
                if step < STEP - 1:
                    xnb = pools["blk"].tile([128, SB, D], BF16, tag="xnb")
                    nc.vector.tensor_copy(xnb[:, :nb, :], xn[:, :nb, :])
                    nc.sync.dma_start(x_mid.ap()[:, b0:b0 + nb, :],
                                      xn[:, :nb, :])
                    for j, b in enumerate(blocks):
                        tp = pC.tile([D, 128], BF16, tag="pj")
                        nc.tensor.transpose(tp[:], xnb[:, j, :], ident_t[:])
                        xts = pools["sm"].tile([D, 128], BF16, tag="xts")
                        nc.scalar.activation(xts[:], tp[:], AF.Copy)
                        nc.sync.dma_start(
                            xT_sh.ap()[:, 128 * b:128 * (b + 1)], xts[:])
                else:
                    for j, b in enumerate(blocks):
                        w = tails.get(b, 128)
                        nc.sync.dma_start(x_out.ap()[128 * b:128 * b + w, :],
                                          xn[:w, j, :])
            assert call_i == len(meta["calls"]) and \
                group_i == len(meta["groups"])

            if step == 0 and STEP > 1 and not SKIP_COLL:
                nc.gpsimd.collective_compute(
                    "AllGather", ALU.bypass,
                    replica_groups=[list(range(N_CORES))],
                    ins=[xT_sh.ap()[:]], outs=[xT_ag.ap()[:]])

    _split_multi_waits(nc)
    lower_extended_insts(nc)
    return nc


# ----------------------------------------------------------------------------
# entry point
# ----------------------------------------------------------------------------

def kernel(x, x0, src, dst, W, attn_l, attn_r, alpha, lamda, **kw):
    global _last_results
    x = np.asarray(x, np.float32)
    x0 = np.asarray(x0, np.float32)
    src = np.asarray(src)
    dst = np.asarray(dst)
    W = np.asarray(W, np.float32)
    attn_l = np.asarray(attn_l, np.float32)
    attn_r = np.asarray(attn_r, np.float32)
    alpha_f = float(np.asarray(alpha))
    lamda_f = float(np.asarray(lamda))

    N, D = x.shape
    H = attn_l.shape[0]
    assert N % N_CORES == 0
    meta = _plan_and_arrays(src, dst, N)
    Nl, NB = meta["Nl"], meta["NB"]
    NBP = NB * 128

    nc = _build(meta, N, D, H)

    # host-side weight prep
    W3 = W.reshape(D, H, D)
    WL = np.einsum("khd,hd->kh", W3, attn_l)
    WR = np.einsum("khd,hd->kh", W3, attn_r)
    waug = _bf(np.concatenate([W, WL], axis=1))
    wr = _bf(WR)
    iota = _bf(np.tile(np.arange(128, dtype=np.float32)[None, :], (128, 1)))
    itld = _bf((32 * (np.arange(128)[:, None] // 32)
                + np.arange(32)[None, :]).astype(np.float32))
    ident = _bf(np.eye(128, dtype=np.float32))
    scal = np.zeros((128, 4), np.float32)
    scal[:, 0] = 1.0 - alpha_f
    scal[:, 1] = alpha_f / H
    c0 = (alpha_f * lamda_f) * x0

    xT = _bf(x.T).copy()                      # [D, N]
    in_maps = []
    for p in range(N_CORES):
        lo = p * Nl
        xl = np.zeros((NBP, D), np.float32)
        xl[:Nl] = x[lo:lo + Nl]
        c0l = np.zeros((NBP, D), np.float32)
        c0l[:Nl] = c0[lo:lo + Nl]
        in_maps.append({
            "xT_in": np.ascontiguousarray(xT),
            "xTl_in": np.ascontiguousarray(_bf(xl.T)),
            "x_in": np.ascontiguousarray(
                xl.reshape(NB, 128, D).transpose(1, 0, 2)),
            "c0_in": np.ascontiguousarray(
                c0l.reshape(NB, 128, D).transpose(1, 0, 2)),
            "waug_in": waug, "wr_in": wr,
            "iota_in": iota, "itld_in": itld, "ident_in": ident,
            "scal_in": scal,
            "idx_in": np.ascontiguousarray(
                np.tile(meta["idx_wrapped"][p], (8, 1))),
            "doff_in": np.ascontiguousarray(_bf(meta["dstoff"][p])),
            "drep_in": np.ascontiguousarray(_bf(meta["dstrep"][p])),
        })

    global _last_nc, _last_in_maps
    _last_nc = nc
    _last_in_maps = in_maps
    trace = bool(int(os.environ.get("GAT_TRACE", "0")))
    res = run_bass_kernel_spmd(nc, in_maps, core_ids=list(range(N_CORES)),
                               trace=trace,
                               trace_cores=list(range(N_CORES)) if trace else None,
                               stitch_traces=False)
    _last_results = res
    out = np.concatenate([res.results[p]["x_out"] for p in range(N_CORES)],
                         axis=0)
    return out.astype(np.float32)



# revision 26
# speedup vs baseline: 1.3122x; 1.3122x over previous
"""GAT (graph attention) message-passing kernel for Trainium2, 8 NeuronCores.

Strategy (graph/data parallel, dst-sharded):
  - Nodes are partitioned across 8 cores by destination id (12500 each).
  - Edges are sharded by dst partition, sorted by (dst-block, src-subtable),
    and padded so every core runs an identical (SPMD) program.
  - The attention left-term el is folded into the projection by a per-head
    Householder change of basis: g = T_h h with g[...,0] = el, stored
    head-minor so table rows are exactly [g interleaved (d,h)] = 512B bf16.
    The epilogue un-rotates with a 256x64 matmul per dst block.
  - Per step, every core projects ALL nodes into its HBM row table, then
    indirect-gathers g[src] rows per edge (dma_gather), builds one-hot dst
    masks on DVE (4x tensor-scalar forms where layouts allow), computes
    attention scores (er via a small maskT matmul), and accumulates
    [softmax-denominator | weighted message sum] into per-dst-block PSUM
    with mask matmuls on TensorE.
  - Block epilogue: normalize by the segment sum, un-rotate + head-mean via
    TensorE, residual update.
  - Between the 2 conv steps, the updated x (transposed, bf16) is AllGathered
    across the 8 cores.
"""

import os
import math
import numpy as np
import ml_dtypes

import concourse.bass as bass
import concourse.tile as tile
import concourse.mybir as mybir
from concourse import library_config
from concourse.library_overlay import lower_extended_insts
from concourse.bass_utils import run_bass_kernel_spmd

BF16 = mybir.dt.bfloat16
F32 = mybir.dt.float32
I16 = mybir.dt.int16
AF = mybir.ActivationFunctionType
ALU = mybir.AluOpType

NEG_SLOPE = 0.2
STEP = int(os.environ.get("GAT_STEPS", "2"))
SKIP_COLL = bool(int(os.environ.get("GAT_SKIP_COLL", "0")))
SKIP_GATHER = bool(int(os.environ.get("GAT_SKIP_GATHER", "0")))
N_CORES = 8
SB = 4            # blocks per superblock (PSUM accumulators alive at once)
MAX_CALL = 32     # max 128-edge chunks per dma_gather call
GS = 16           # chunks per elementwise batch group
ST_MAX_ROWS = 25000   # subtable rows (int16 gather index limit)

_last_results = None  # BassKernelResults stash for test harness
_last_nc = None       # built Bass module (for test-side benching)
_last_in_maps = None  # per-core input maps (for test-side benching)


def _bf(x):
    return np.asarray(x, np.float32).astype(ml_dtypes.bfloat16)


# ----------------------------------------------------------------------------
# host-side preprocessing
# ----------------------------------------------------------------------------

def _plan_and_arrays(src, dst, N):
    """Shard/sort/pad edges; build the shared chunk plan and per-core arrays."""
    Nl = N // N_CORES
    NB = (Nl + 127) // 128
    NSB = (NB + SB - 1) // SB
    NST = max(1, math.ceil(N / ST_MAX_ROWS))
    st_rows = math.ceil(N / NST)

    core = dst // Nl
    percore = []
    for p in range(N_CORES):
        sel = np.nonzero(core == p)[0]
        s = src[sel].astype(np.int64)
        d = (dst[sel] - p * Nl).astype(np.int64)
        blk = d >> 7
        st = s // st_rows
        order = np.lexsort((s, st, blk))
        percore.append((s[order], d[order], blk[order], st[order]))

    counts = np.zeros((N_CORES, NB, NST), np.int64)
    for p in range(N_CORES):
        _, _, blk, st = percore[p]
        np.add.at(counts, (p, blk, st), 1)
    nchunks = (counts.max(axis=0) + 127) // 128          # [NB, NST]

    # canonical chunk emission order
    chunk_meta = []   # (isb, st, b) per chunk
    calls = []        # (st, chunk_lo, n_chunks)
    for isb in range(NSB):
        blocks = range(isb * SB, min((isb + 1) * SB, NB))
        for st in range(NST):
            run_lo = len(chunk_meta)
            for b in blocks:
                for _ in range(int(nchunks[b, st])):
                    chunk_meta.append((isb, st, b))
            n = len(chunk_meta) - run_lo
            o = run_lo
            while n > 0:
                take = min(n, MAX_CALL)
                calls.append((st, o, take))
                o += take
                n -= take
    NCH = len(chunk_meta)

    # first/last chunk index per (isb, b) for PSUM start/stop flags
    first = {}
    last = {}
    for ci, (isb, st, b) in enumerate(chunk_meta):
        key = (isb, b)
        if key not in first:
            first[key] = ci
        last[key] = ci

    # per-core edge arrays in padded chunk order
    idx_all = np.zeros((N_CORES, NCH * 128), np.int16)
    doff_all = np.full((N_CORES, NCH * 128), 255.0, np.float32)
    for p in range(N_CORES):
        s, d, blk, st = percore[p]
        # build run boundaries of the (blk, st)-sorted edge list
        runs = {}
        i = 0
        M = len(s)
        while i < M:
            k = (blk[i], st[i])
            j = i
            while j < M and blk[j] == k[0] and st[j] == k[1]:
                j += 1
            runs[k] = (i, j)
            i = j
        cursor = {k: v[0] for k, v in runs.items()}
        for ci, (isb, t, b) in enumerate(chunk_meta):
            base = ci * 128
            k = (b, t)
            if k in runs:
                lo = cursor[k]
                hi = min(lo + 128, runs[k][1])
                n = hi - lo
                cursor[k] = hi
                if n > 0:
                    idx_all[p, base:base + n] = (s[lo:hi] - t * st_rows).astype(np.int16)
                    doff_all[p, base:base + n] = (d[lo:hi] - b * 128).astype(np.float32)
        for k, (lo, hi) in runs.items():
            assert cursor[k] == hi, "edge run not fully consumed"

    # gather-call wrapped idx layout: per call [16, n/16], concat on free axis
    idxw_cols = NCH * 8
    idx_wrapped = np.zeros((N_CORES, 16, idxw_cols), np.int16)
    col = 0
    call_cols = []
    for (t, lo, nch) in calls:
        n = nch * 128
        for p in range(N_CORES):
            seg = idx_all[p, lo * 128: lo * 128 + n]
            idx_wrapped[p, :, col:col + n // 16] = seg.reshape(-1, 16).T
        call_cols.append(col)
        col += n // 16
    assert col == idxw_cols

    # dstoff duplicated pairs [128, 2*NCH] so the one-hot build's broadcast
    # has a unit-stride last dim (DVE 4x mode): doff2[p, 2c+j] = doff(c, p)
    doff = doff_all.reshape(N_CORES, NCH, 128).transpose(0, 2, 1)  # [p,128,NCH]
    doff2 = np.repeat(doff, 2, axis=2)                             # [p,128,2NCH]
    groups = []
    for (t, lo, nch) in calls:
        g = lo
        while g < lo + nch:
            take = min(GS, lo + nch - g)
            groups.append((t, lo, g, take))  # (st, call_lo, group_lo, size)
            g += take

    return dict(Nl=Nl, NB=NB, NSB=NSB, NST=NST, st_rows=st_rows, NCH=NCH,
                chunk_meta=chunk_meta, calls=calls, call_cols=call_cols,
                groups=groups, first=first, last=last,
                idx_wrapped=idx_wrapped, doff2=doff2,
                idxw_cols=idxw_cols)



def _balance_partition(src, dst, N):
    """Quartile-preserving node re-partition: pack nodes into (core, block)
    bins so per-(block, subtable) edge counts stay under 128-multiple quotas
    (minimizes gather-chunk padding and equalizes cores). Returns perm with
    perm[orig_id] = new_id; new id stays inside the node's src-subtable."""
    Nl = N // N_CORES
    NB = (Nl + 127) // 128
    NST = max(1, math.ceil(N / ST_MAX_ROWS))
    st_rows = math.ceil(N / NST)
    if st_rows % Nl != 0:
        return np.arange(N, dtype=np.int64)
    CPQ = st_rows // Nl
    NBINS = CPQ * NB
    st_arr = src // st_rows
    deg4 = np.zeros((N, NST), np.int32)
    np.add.at(deg4, (dst, st_arr), 1)

    perm = np.empty(N, np.int64)
    for q in range(NST):
        nodes = np.arange(q * st_rows, min((q + 1) * st_rows, N))
        d4 = deg4[nodes].astype(np.float64)
        order = np.argsort(-d4.sum(1), kind="stable")
        cap_n = np.full(NBINS, 128, np.int64)
        for c in range(CPQ):
            cap_n[c * NB + NB - 1] = Nl - 128 * (NB - 1)
        # per-(core, st) quotas in multiples of 128
        cap_e = np.zeros((NBINS, NST))
        tot_s = d4.sum(0) / CPQ                      # per-core totals
        for c in range(CPQ):
            sl = slice(c * NB, (c + 1) * NB)
            qb = tot_s[None, :] * (cap_n[sl, None] / cap_n[sl].sum())
            base = np.floor(qb / 128).astype(np.int64)
            caps = base.copy()
            for s in range(NST):
                need = int(math.ceil(
                    (tot_s[s] * 1.01 + 256 - 128 * base[:, s].sum()) / 128))
                if need > 0:
                    o = np.argsort(-(qb[:, s] / 128 - base[:, s]))
                    for i in range(need):
                        caps[o[i % NB], s] += 1
            cap_e[sl] = caps * 128.0
        sums = np.zeros((NBINS, NST))
        cnt = np.zeros(NBINS, np.int64)
        assign = np.empty(len(nodes), np.int64)
        for i in order:
            v = d4[i]
            ns = sums + v
            feasible = (ns <= cap_e).all(axis=1) & (cnt < cap_n)
            if feasible.any():
                score = (ns / np.maximum(cap_e, 1)).max(axis=1)
                score[~feasible] = np.inf
                b = int(np.argmin(score))
            else:
                over = np.maximum(ns - cap_e, 0).sum(axis=1)
                over[cnt >= cap_n] = np.inf
                b = int(np.argmin(over))
            assign[i] = b
            sums[b] += v
            cnt[b] += 1
        for _ in range(60):
            bad = np.argwhere(sums > cap_e)
            if len(bad) == 0:
                break
            fixed = 0
            for b, s in bad:
                while sums[b, s] > cap_e[b, s]:
                    members = np.where(assign == b)[0]
                    need = sums[b, s] - cap_e[b, s]
                    md = d4[members, s]
                    okm = members[md >= need]
                    mi = okm[np.argmin(d4[okm, s])] if len(okm) else \
                        members[np.argmax(md)]
                    vout = d4[mi]
                    slack = cap_e - sums
                    ok = (slack[:, s] >= vout[s])
                    ok[b] = False
                    if not ok.any():
                        break
                    done = False
                    cb = np.nonzero(ok)[0]
                    for tb in cb[np.argsort(-slack[cb, s])][:16]:
                        tmem = np.where(assign == tb)[0]
                        vd = d4[tmem]
                        newA = sums[b] - vout + vd
                        newB = sums[tb] + vout - vd
                        okv = (newA <= cap_e[b]).all(axis=1) & \
                            (newB <= cap_e[tb]).all(axis=1)
                        if not okv.any():
                            continue
                        cand = tmem[okv]
                        vi = cand[np.argmin(d4[cand, s])]
                        assign[mi], assign[vi] = tb, b
                        sums[b] += d4[vi] - vout
                        sums[tb] += vout - d4[vi]
                        fixed += 1
                        done = True
                        break
                    if not done:
                        break
            if fixed == 0:
                break
        for b in range(NBINS):
            members = nodes[assign == np.int64(b)]
            core = q * CPQ + b // NB
            blk = b % NB
            base = core * Nl + 128 * blk
            perm[members] = base + np.arange(len(members))
    return perm


# ----------------------------------------------------------------------------
# device program
# ----------------------------------------------------------------------------

def _split_multi_waits(nc):
    """walrus codegen only accepts one sync-wait per instruction; hoist any
    extra waits onto same-engine NOPs inserted right before the instruction."""
    n_id = 0
    for f in nc.m.functions:
        for blk in f.blocks:
            out = []
            for ins in blk.instructions:
                si = ins.sync_info
                if si is not None and len(si.on_wait) > 1 \
                        and ins.engine is not None:
                    waits = list(si.on_wait)
                    for w in waits[:-1]:
                        nop = mybir.InstNoOp(name=f"I-wsplit-{n_id}", ins=[],
                                             outs=[])
                        n_id += 1
                        nop.engine = ins.engine
                        nop.sync_info = mybir.SyncInfo(on_wait=[w],
                                                       on_update=[])
                        nc.inst_map[nop.name] = nop
                        out.append(nop)
                    ins.sync_info = mybir.SyncInfo(on_wait=[waits[-1]],
                                                   on_update=list(si.on_update))
                out.append(ins)
            blk.instructions = out

def _ap(base, *dims):
    """Rebuild AP with the same tensor/offset/partition dim, custom free dims."""
    return bass.AP(base.tensor, base.offset,
                   [list(base.ap[0])] + [list(d) for d in dims])


def _build(meta, N, D, H):
    Nl, NB, NSB, NST = meta["Nl"], meta["NB"], meta["NSB"], meta["NST"]
    st_rows = meta["st_rows"]
    NBP = NB * 128
    HD = H * D            # 256
    TW = HD               # table row: g head-minor, 512B bf16
    RW = HD + H           # 260: [exp | weighted msg] accumulate width

    nc = bass.Bass("TRN2", target_bir_lowering=False, debug=False,
                   enable_asserts=False, num_devices=N_CORES)

    # ---- DRAM tensors
    xT_in = nc.dram_tensor("xT_in", [D, N], BF16, kind="ExternalInput")
    xTl_in = nc.dram_tensor("xTl_in", [D, NBP], BF16, kind="ExternalInput")
    xc_in = nc.dram_tensor("xc_in", [128, NB, 2, D], F32, kind="ExternalInput")
    waug_in = nc.dram_tensor("waug_in", [D, TW], BF16, kind="ExternalInput")
    wr_in = nc.dram_tensor("wr_in", [D, H], BF16, kind="ExternalInput")
    mbig_in = nc.dram_tensor("mbig_in", [128, 2, D], BF16, kind="ExternalInput")
    iota_in = nc.dram_tensor("iota_in", [128, 128], BF16, kind="ExternalInput")
    ident_in = nc.dram_tensor("ident_in", [128, 128], BF16, kind="ExternalInput")
    scal_in = nc.dram_tensor("scal_in", [128, 4], F32, kind="ExternalInput")
    idx_in = nc.dram_tensor("idx_in", [128, meta["idxw_cols"]], I16,
                            kind="ExternalInput")
    doff2_in = nc.dram_tensor("doff2_in", [128, 2 * meta["NCH"]], BF16,
                              kind="ExternalInput")

    # one table tensor per src-subtable so gathers of subtable q only wait
    # on that quarter's projection writes (projection/gather overlap)
    split_tbl = (NST > 1 and st_rows % Nl == 0)
    ntbl = NST if split_tbl else 1
    tables = [nc.dram_tensor(f"table{q}",
                             [min(st_rows, N - q * st_rows) if split_tbl else N,
                              TW], BF16, kind="Internal")
              for q in range(ntbl)]
    xp_mid = nc.dram_tensor("xp_mid", [128, NB, D], F32, kind="Internal")
    xT_sh = nc.dram_tensor("xT_sh", [D, NBP], BF16, kind="Internal")
    xT_ag = nc.dram_tensor("xT_ag", [D * N_CORES, NBP], BF16, kind="Internal",
                           addr_space="Shared")
    x_out = nc.dram_tensor("x_out", [Nl, D], F32, kind="ExternalOutput")

    from contextlib import ExitStack
    with tile.TileContext(nc) as tc, ExitStack() as es_:
        nc.gpsimd.load_library(library_config.mlp)
        cp = es_.enter_context(tc.tile_pool(name="consts", bufs=1))
        pools = {}
        for nm, bufs in [("xt", 4), ("rows", 2), ("mask", 2), ("rhs", 3),
                         ("sm", 3), ("tbl", 4), ("blk", 3), ("big", 2)]:
            pools[nm] = es_.enter_context(tc.tile_pool(name=nm, bufs=bufs))
        pA = es_.enter_context(tc.tile_pool(name="pacc", bufs=1, space="PSUM"))
        pB = es_.enter_context(tc.tile_pool(name="per8", bufs=2, space="PSUM"))
        pC = es_.enter_context(tc.tile_pool(name="ppj", bufs=2, space="PSUM"))

        # ---- load constants
        iota_t = cp.tile([128, 128], BF16, tag="iota")
        ident_t = cp.tile([128, 128], BF16, tag="ident")
        waug_t = cp.tile([D, TW], BF16, tag="waug")
        wr_t = cp.tile([D, H], BF16, tag="wr")
        mbig_t = cp.tile([128, 2, D], BF16, tag="mbig")
        scal_t = cp.tile([128, 4], F32, tag="scal")
        idx_t = cp.tile([128, meta["idxw_cols"]], I16, tag="idx")
        doff2_t = cp.tile([128, 2 * meta["NCH"]], BF16, tag="doff2")
        for t, s in [(iota_t, iota_in), (ident_t, ident_in),
                     (waug_t, waug_in), (wr_t, wr_in), (mbig_t, mbig_in),
                     (scal_t, scal_in), (idx_t, idx_in), (doff2_t, doff2_in)]:
            nc.sync.dma_start(t[:], s.ap()[:])

        tails = {NB - 1: Nl - 128 * (NB - 1)}
        nidx_regs = {}

        def nidx_reg(n):
            if n not in nidx_regs:
                nidx_regs[n] = nc.gpsimd.to_reg(n)
            return nidx_regs[n]

        for step in range(STEP):
            # ------------------------------------------------ projection
            # batched: one [D, 512] load + one 512-row table write per 4 blocks
            eng_flip = 0
            PB = 8
            for r in range(N_CORES):
                for t0 in range(0, NB, PB):
                    bts = list(range(t0, min(t0 + PB, NB)))
                    o = 128 * t0
                    w = min(128 * PB, Nl - o)
                    g0 = r * Nl + o
                    xt = pools["xt"].tile([D, 128 * PB], BF16, tag="projlhs")
                    if step == 0:
                        nc.gpsimd.dma_start(xt[:, :w],
                                            xT_in.ap()[:, g0:g0 + w])
                    else:
                        nc.gpsimd.dma_start(
                            xt[:, :w], xT_ag.ap()[D * r:D * (r + 1), o:o + w])
                    tb4 = pools["tbl"].tile([128, PB, TW], BF16, tag="tbl")
                    q = (g0 // st_rows) if split_tbl else 0
                    tq = tables[q]
                    gq = g0 - q * st_rows if split_tbl else g0
                    nfull = 0
                    part = None
                    for j, t in enumerate(bts):
                        wj = min(128, Nl - 128 * t)
                        scr = pC.tile([128, 512], F32, tag="scr")
                        nc.tensor.matmul(scr[:wj, 0:TW],
                                         xt[:, 128 * j:128 * j + wj],
                                         waug_t[:], start=True, stop=True)
                        if eng_flip % 2 == 0:
                            nc.vector.tensor_copy(tb4[:wj, j, :],
                                                  scr[:wj, 0:TW])
                        else:
                            nc.scalar.activation(tb4[:wj, j, :],
                                                 scr[:wj, 0:TW], AF.Copy)
                        eng_flip += 1
                        if wj == 128:
                            nfull += 1
                        else:
                            part = (j, wj)
                    if nfull:
                        dst = bass.AP(tq.ap().tensor, gq * TW,
                                      [[TW, 128], [128 * TW, nfull], [1, TW]])
                        nc.sync.dma_start(dst, tb4[:, :nfull, :])
                    if part is not None:
                        j, wj = part
                        gp = gq + 128 * j
                        nc.sync.dma_start(tq.ap()[gp:gp + wj, :],
                                          tb4[:wj, j, :])

            # ------------------------------------------------ gather + attn
            xt_src = xTl_in if step == 0 else xT_sh
            call_i = 0
            group_i = 0
            for isb in range(NSB):
                blocks = list(range(isb * SB, min((isb + 1) * SB, NB)))
                nb = len(blocks)
                b0 = blocks[0]
                acc = pA.tile([128, SB, 512], F32, tag="acc")
                # x4p = (1-alpha)*x + alpha*lamda*x0 — precombined for step>0
                x4p = pools["blk"].tile([128, SB, D], F32, tag="x4p")
                if step == 0:
                    xc4 = pools["blk"].tile([128, SB, 2, D], F32, tag="xc4")
                    nc.gpsimd.dma_start(xc4[:, :nb, :, :],
                                        xc_in.ap()[:, b0:b0 + nb, :, :])
                    nc.vector.scalar_tensor_tensor(
                        x4p[:, :nb, :], xc4[:, :nb, 0, :], scal_t[:, 0:1],
                        xc4[:, :nb, 1, :], op0=ALU.mult, op1=ALU.add)
                else:
                    nc.gpsimd.dma_start(x4p[:, :nb, :],
                                        xp_mid.ap()[:, b0:b0 + nb, :])
                # er for the superblock: one [D, SB*128] load + per-block matmul
                wsb = min(SB * 128, NBP - 128 * b0)
                xtb4 = pools["xt"].tile([D, SB * 128], BF16, tag="erlhs")
                nc.gpsimd.dma_start(
                    xtb4[:, :wsb], xt_src.ap()[:, 128 * b0:128 * b0 + wsb])
                es4 = pools["sm"].tile([128, SB, H], BF16, tag="er4")
                for j, b in enumerate(blocks):
                    nc.tensor.matmul(acc[:, j, 264:264 + H],
                                     xtb4[:, 128 * j:128 * (j + 1)], wr_t[:],
                                     start=True, stop=True)
                    nc.scalar.activation(es4[:, j, :], acc[:, j, 264:264 + H],
                                         AF.Copy)

                # walk this superblock's calls/groups/chunks
                while call_i < len(meta["calls"]):
                    st, lo, nch = meta["calls"][call_i]
                    if lo >= len(meta["chunk_meta"]) or \
                       meta["chunk_meta"][lo][0] != isb:
                        break
                    n = nch * 128
                    rows = pools["rows"].tile([128, MAX_CALL, TW], BF16,
                                              tag="rows")
                    icol = meta["call_cols"][call_i]
                    rows_ap = _ap(rows[:], [TW, nch], [1, TW])
                    if split_tbl:
                        tbl_ap = tables[st].ap()[:, :]
                    else:
                        tbl_ap = tables[0].ap()[st * st_rows:
                                                min((st + 1) * st_rows, N), :]
                    if not SKIP_GATHER:
                        nc.gpsimd.dma_gather(
                            rows_ap, tbl_ap, idx_t[:, icol:icol + n // 16],
                            num_idxs=n, num_idxs_reg=nidx_reg(n), elem_size=TW,
                            single_packet=False)
                    call_i += 1

                    while group_i < len(meta["groups"]):
                        gst, glo_call, g, gs = meta["groups"][group_i]
                        if glo_call != lo:
                            break
                        group_i += 1
                        cc0 = g - lo   # chunk offset within call
                        # one-hot dst mask [128e, gs, 128n] — 2x TT form
                        # (doff2 pair-duplication keeps last dim unit-stride)
                        m8 = pools["mask"].tile([128, GS, 128], BF16, tag="m8")
                        nc.vector.tensor_tensor(
                            _ap(m8[:], [128, gs], [2, 64], [1, 2]),
                            _ap(iota_t[:], [0, gs], [2, 64], [1, 2]),
                            _ap(doff2_t[:, 2 * g:2 * (g + gs)],
                                [2, gs], [0, 64], [1, 2]),
                            op=ALU.is_equal)
                        # mT8: per-chunk PE transposes into PSUM (8-chunk
                        # batches), then batched copies out (alternate DVE/Act)
                        mT8 = pools["mask"].tile([128, GS, 128], BF16, tag="mT8")
                        for h0 in range(0, gs, 8):
                            hn = min(8, gs - h0)
                            ms = pB.tile([128, 512], F32, tag="scr8")
                            for k in range(hn):
                                nc.tensor.transpose(
                                    ms[:, 64 * k:64 * (k + 1)].bitcast(BF16),
                                    m8[:, h0 + k, :], ident_t[:])
                            if (group_i + h0) % 2 == 0:
                                nc.vector.tensor_copy(
                                    _ap(mT8[:, h0:h0 + hn, :], [1, hn * 128]),
                                    ms[:, 0:64 * hn].bitcast(BF16))
                            else:
                                nc.scalar.activation(
                                    _ap(mT8[:, h0:h0 + hn, :], [1, hn * 128]),
                                    ms[:, 0:64 * hn].bitcast(BF16), AF.Copy)
                        er8 = pB.tile([128, 512], F32, tag="scr8")
                        for k in range(gs):
                            ci = g + k
                            _, _, b = meta["chunk_meta"][ci]
                            j = b - b0
                            nc.tensor.matmul(er8[:, H * k:H * (k + 1)],
                                             mT8[:, k, :], es4[:, j, :],
                                             start=True, stop=True)
                        # t8 = er[dst] + el[src] (el = first H cols of rows)
                        t8 = pools["sm"].tile([128, GS * H], BF16, tag="t8")
                        nc.vector.tensor_tensor(
                            _ap(t8[:], [H, gs], [1, H]),
                            _ap(er8[:], [H, gs], [1, H]),
                            _ap(rows[:, cc0:cc0 + gs, 0:H], [TW, gs], [1, H]),
                            op=ALU.add)
                        lr8 = pools["sm"].tile([128, GS * H], BF16, tag="lr8")
                        nc.vector.scalar_tensor_tensor(
                            lr8[:, :gs * H], t8[:, :gs * H], scal_t[:, 3:4],
                            t8[:, :gs * H], op0=ALU.mult, op1=ALU.max)
                        rhs8 = pools["rhs"].tile([128, GS, RW], BF16, tag="rhs8")
                        nc.scalar.activation(
                            _ap(rhs8[:], [RW, gs], [1, H]),
                            _ap(lr8[:], [H, gs], [1, H]), AF.Exp)
                        # msg = g_row * exp — 2x TT form (head-minor layout
                        # keeps the exp broadcast's last dim unit-stride)
                        nc.vector.tensor_tensor(
                            _ap(rhs8[:, :, H:RW], [RW, gs], [H, D], [1, H]),
                            _ap(rows[:, cc0:cc0 + gs, :],
                                [TW, gs], [H, D], [1, H]),
                            _ap(rhs8[:], [RW, gs], [0, D], [1, H]),
                            op=ALU.mult)
                        for k in range(gs):
                            ci = g + k
                            _, _, b = meta["chunk_meta"][ci]
                            j = b - b0
                            nc.tensor.matmul(
                                acc[:, j, 0:RW], m8[:, k, :], rhs8[:, k, :],
                                start=(meta["first"][(isb, b)] == ci),
                                stop=(meta["last"][(isb, b)] == ci),
                                skip_group_check=True)

                # ---- superblock epilogue (batched over blocks)
                smax = pools["sm"].tile([128, SB * H], F32, tag="smax")
                nc.vector.tensor_scalar(
                    _ap(smax[:], [H, nb], [1, H]),
                    _ap(acc[:], [512, nb], [1, H]),
                    1e-30, None, op0=ALU.max)
                srec = pools["sm"].tile([128, SB * H], F32, tag="srec")
                nc.vector.reciprocal(srec[:, :nb * H], smax[:, :nb * H])
                srec2 = pools["sm"].tile([128, SB * H], F32, tag="srec2")
                nc.vector.tensor_scalar(
                    srec2[:, :nb * H], srec[:, :nb * H], scal_t[:, 1:2], None,
                    op0=ALU.mult)
                # normalized messages, bf16, head-minor [n, (d,h)]
                onorm = pools["big"].tile([128, SB, HD], BF16, tag="onorm")
                nc.vector.tensor_tensor(
                    _ap(onorm[:], [HD, nb], [H, D], [1, H]),
                    _ap(acc[:, :, H:RW], [512, nb], [H, D], [1, H]),
                    _ap(srec2[:], [H, nb], [0, D], [1, H]),
                    op=ALU.mult)
                xn = pools["blk"].tile([128, SB, D], F32, tag="xn")
                for j, b in enumerate(blocks):
                    # un-rotate + head-mean: out = onorm_j @ Mbig  (256->64)
                    scr = pC.tile([128, 512], F32, tag="scr")
                    nc.tensor.transpose(scr[:, 256:320].bitcast(BF16),
                                        onorm[:, j, 0:128], ident_t[:])
                    nc.tensor.transpose(scr[:, 320:384].bitcast(BF16),
                                        onorm[:, j, 128:256], ident_t[:])
                    oTs = pools["big"].tile([128, 2, 128], BF16, tag="oTs")
                    nc.scalar.activation(_ap(oTs[:], [1, 256]),
                                         scr[:, 256:384].bitcast(BF16),
                                         AF.Copy)
                    red = pB.tile([128, 512], F32, tag="scr8")
                    nc.tensor.matmul(red[:, 0:D], oTs[:, 0, :], mbig_t[:, 0, :],
                                     start=True, stop=False)
                    nc.tensor.matmul(red[:, 0:D], oTs[:, 1, :], mbig_t[:, 1, :],
                                     start=False, stop=True)
                    nc.vector.tensor_add(xn[:, j, :], x4p[:, j, :],
                                         red[:, 0:D])
                if step < STEP - 1:
                    if step > 0:
                        xc4 = pools["blk"].tile([128, SB, 2, D], F32,
                                                tag="xc4")
                        nc.sync.dma_start(
                            xc4[:, :nb, 1, :],
                            xc_in.ap()[:, b0:b0 + nb, 1, :])
                    xp = pools["blk"].tile([128, SB, D], F32, tag="xp")
                    nc.vector.scalar_tensor_tensor(
                        xp[:, :nb, :], xn[:, :nb, :], scal_t[:, 0:1],
                        xc4[:, :nb, 1, :], op0=ALU.mult, op1=ALU.add)
                    nc.sync.dma_start(xp_mid.ap()[:, b0:b0 + nb, :],
                                      xp[:, :nb, :])
                    xnb = pools["blk"].tile([128, SB, D], BF16, tag="xnb")
                    nc.vector.tensor_copy(xnb[:, :nb, :], xn[:, :nb, :])
                    xts4 = pools["sm"].tile([D, SB, 128], BF16, tag="xts")
                    for j, b in enumerate(blocks):
                        scr = pC.tile([128, 512], F32, tag="scr")
                        nc.tensor.transpose(scr[:D, 0:64].bitcast(BF16),
                                            xnb[:, j, :], ident_t[:])
                        nc.scalar.activation(xts4[:, j, :],
                                             scr[:D, 0:64].bitcast(BF16),
                                             AF.Copy)
                    nc.sync.dma_start(
                        xT_sh.ap()[:, 128 * b0:128 * (b0 + nb)],
                        xts4[:, :nb, :])
                else:
                    for j, b in enumerate(blocks):
                        w = tails.get(b, 128)
                        nc.sync.dma_start(x_out.ap()[128 * b:128 * b + w, :],
                                          xn[:w, j, :])
            assert call_i == len(meta["calls"]) and \
                group_i == len(meta["groups"])

            if step == 0 and STEP > 1 and not SKIP_COLL:
                nc.gpsimd.collective_compute(
                    "AllGather", ALU.bypass,
                    replica_groups=[list(range(N_CORES))],
                    ins=[xT_sh.ap()[:]], outs=[xT_ag.ap()[:]])

    _split_multi_waits(nc)
    lower_extended_insts(nc)
    return nc


# ----------------------------------------------------------------------------
# entry point
# ----------------------------------------------------------------------------

def _fold_weights(W, attn_l, D, H):
    """Per-head change of basis T_h = D_h @ Householder_h with g = T_h h,
    g[0] = attn_l[h]·h exactly. Returns (waug head-minor [D, H*D],
    Mbig [H*D, D] un-rotation, head-minor rows)."""
    W3 = W.reshape(D, H, D).astype(np.float64)
    waug_hm = np.empty((D, H, D))   # [k, h, d] -> col (d*H + h)
    mbig = np.empty((H, D, D))
    for h in range(H):
        a = attn_l[h].astype(np.float64)
        norm = np.linalg.norm(a)
        v = a / norm
        sign = 1.0 if v[0] >= 0 else -1.0
        u = v.copy()
        u[0] += sign
        u /= np.linalg.norm(u)
        House = np.eye(D) - 2.0 * np.outer(u, u)   # maps v -> -sign*e0
        T = House.copy()
        T[0, :] *= -sign * norm                    # D_h @ House: g[0] = a·h
        waug_hm[:, h, :] = W3[:, h, :] @ T.T
        Minv = House.copy()
        Minv[0, :] *= 1.0 / (-sign * norm)         # Mbig = Dinv @ House:
        mbig[h] = Minv                             # out[n,j] = Σ_k g[n,k]·Mbig[k,j]
    # head-minor interleave: waug[:, d*H + h]
    waug = waug_hm.transpose(0, 2, 1).reshape(D, H * D)
    mbig_hm = mbig.transpose(1, 0, 2).reshape(H * D, D)  # row (d*H+h) -> [D]
    return waug, mbig_hm


def kernel(x, x0, src, dst, W, attn_l, attn_r, alpha, lamda, **kw):
    global _last_results
    x = np.asarray(x, np.float32)
    x0 = np.asarray(x0, np.float32)
    src = np.asarray(src)
    dst = np.asarray(dst)
    W = np.asarray(W, np.float32)
    attn_l = np.asarray(attn_l, np.float32)
    attn_r = np.asarray(attn_r, np.float32)
    alpha_f = float(np.asarray(alpha))
    lamda_f = float(np.asarray(lamda))

    N, D = x.shape
    H = attn_l.shape[0]
    assert N % N_CORES == 0
    if bool(int(os.environ.get("GAT_BALANCE", "1"))):
        perm = _balance_partition(src, dst, N)
    else:
        perm = np.arange(N, dtype=np.int64)
    inv = np.argsort(perm)
    x = x[inv]
    x0 = x0[inv]
    src = perm[src]
    dst = perm[dst]
    meta = _plan_and_arrays(src, dst, N)
    Nl, NB = meta["Nl"], meta["NB"]
    NBP = NB * 128

    nc = _build(meta, N, D, H)

    # host-side weight prep
    waug_f, mbig_f = _fold_weights(W, attn_l, D, H)
    W3 = W.reshape(D, H, D)
    WR = np.einsum("khd,hd->kh", W3, attn_r)
    waug = _bf(waug_f)
    wr = _bf(WR)
    mbig = _bf(mbig_f.reshape(2, 128, D).transpose(1, 0, 2))  # [128, 2, D]
    iota = _bf(np.tile(np.arange(128, dtype=np.float32)[None, :], (128, 1)))
    ident = _bf(np.eye(128, dtype=np.float32))
    scal = np.zeros((128, 4), np.float32)
    scal[:, 0] = 1.0 - alpha_f
    scal[:, 1] = alpha_f / H
    scal[:, 3] = NEG_SLOPE
    c0 = (alpha_f * lamda_f) * x0

    xT = _bf(x.T).copy()                      # [D, N]
    in_maps = []
    for p in range(N_CORES):
        lo = p * Nl
        xl = np.zeros((NBP, D), np.float32)
        xl[:Nl] = x[lo:lo + Nl]
        c0l = np.zeros((NBP, D), np.float32)
        c0l[:Nl] = c0[lo:lo + Nl]
        xc = np.stack([xl.reshape(NB, 128, D).transpose(1, 0, 2),
                       c0l.reshape(NB, 128, D).transpose(1, 0, 2)],
                      axis=2)                 # [128, NB, 2, D]
        in_maps.append({
            "xT_in": np.ascontiguousarray(xT),
            "xTl_in": np.ascontiguousarray(_bf(xl.T)),
            "xc_in": np.ascontiguousarray(xc),
            "waug_in": waug, "wr_in": wr, "mbig_in": mbig,
            "iota_in": iota, "ident_in": ident,
            "scal_in": scal,
            "idx_in": np.ascontiguousarray(
                np.tile(meta["idx_wrapped"][p], (8, 1))),
            "doff2_in": np.ascontiguousarray(_bf(meta["doff2"][p])),
        })

    global _last_nc, _last_in_maps
    _last_nc = nc
    _last_in_maps = in_maps
    if os.environ.get("GAT_EXEC", "") == "sim":
        from concourse.bass_interp import MultiCoreSim
        sim = MultiCoreSim(nc, N_CORES, num_workers=N_CORES)
        for p in range(N_CORES):
            for name, arr in in_maps[p].items():
                sim.cores[p].tensor(name)[:] = arr
        sim.simulate()
        out = np.concatenate([np.asarray(sim.cores[p].tensor("x_out"))
                              for p in range(N_CORES)], axis=0)
        return out[perm].astype(np.float32)
    trace = bool(int(os.environ.get("GAT_TRACE", "0")))
    res = run_bass_kernel_spmd(nc, in_maps, core_ids=list(range(N_CORES)),
                               trace=trace,
                               trace_cores=list(range(N_CORES)) if trace else None,
                               stitch_traces=False)
    _last_results = res
    out = np.concatenate([res.results[p]["x_out"] for p in range(N_CORES)],
                         axis=0)
    return out[perm].astype(np.float32)


# revision 37
# speedup vs baseline: 2.0718x; 1.5789x over previous
"""GAT (graph attention) message-passing kernel for Trainium2, 8 NeuronCores.

Strategy (graph/data parallel, dst-sharded):
  - Nodes are partitioned across 8 cores by destination id (12500 each).
  - Edges are sharded by dst partition, sorted by (dst-block, src-subtable),
    and padded so every core runs an identical (SPMD) program.
  - The attention left-term el is folded into the projection by a per-head
    Householder change of basis: g = T_h h with g[...,0] = el, stored
    head-minor so table rows are exactly [g interleaved (d,h)] = 512B bf16.
    The epilogue un-rotates with a 256x64 matmul per dst block.
  - Per step, every core projects ALL nodes into its HBM row table, then
    indirect-gathers g[src] rows per edge (dma_gather), builds one-hot dst
    masks on DVE (4x tensor-scalar forms where layouts allow), computes
    attention scores (er via a small maskT matmul), and accumulates
    [softmax-denominator | weighted message sum] into per-dst-block PSUM
    with mask matmuls on TensorE.
  - Block epilogue: normalize by the segment sum, un-rotate + head-mean via
    TensorE, residual update.
  - Between the 2 conv steps, the updated x (transposed, bf16) is AllGathered
    across the 8 cores.
"""

import os
import math
import numpy as np
import ml_dtypes

import concourse.bass as bass
import concourse.tile as tile
import concourse.mybir as mybir
from concourse import library_config
from concourse.library_overlay import lower_extended_insts
from concourse.bass_utils import run_bass_kernel_spmd

BF16 = mybir.dt.bfloat16
F32 = mybir.dt.float32
I16 = mybir.dt.int16
AF = mybir.ActivationFunctionType
ALU = mybir.AluOpType

NEG_SLOPE = 0.2
STEP = int(os.environ.get("GAT_STEPS", "2"))
SKIP_COLL = bool(int(os.environ.get("GAT_SKIP_COLL", "0")))
SKIP_GATHER = bool(int(os.environ.get("GAT_SKIP_GATHER", "0")))
N_CORES = 8
SB = 4            # blocks per superblock (PSUM accumulators alive at once)
MAX_CALL = int(os.environ.get("GAT_MAXCALL", "16"))
GS = int(os.environ.get("GAT_GS", "16"))
ROWS_BUFS = int(os.environ.get("GAT_ROWSBUFS", "6"))
LOAD_ENG = os.environ.get("GAT_LOADENG", "sync")
MT8_MODE = os.environ.get("GAT_MT8", "pe")
NSWQ = int(os.environ.get("GAT_NSWQ", "4"))
ST_MAX_ROWS = 25000   # subtable rows (int16 gather index limit)

_last_results = None  # BassKernelResults stash for test harness
_last_nc = None       # built Bass module (for test-side benching)
_last_in_maps = None  # per-core input maps (for test-side benching)


def _bf(x):
    return np.asarray(x, np.float32).astype(ml_dtypes.bfloat16)


# ----------------------------------------------------------------------------
# host-side preprocessing
# ----------------------------------------------------------------------------

def _plan_and_arrays(src, dst, N):
    """Shard/sort/pad edges; build the shared chunk plan and per-core arrays."""
    Nl = N // N_CORES
    NB = (Nl + 127) // 128
    NSB = (NB + SB - 1) // SB
    NST = max(1, math.ceil(N / ST_MAX_ROWS))
    st_rows = math.ceil(N / NST)

    core = dst // Nl
    percore = []
    for p in range(N_CORES):
        sel = np.nonzero(core == p)[0]
        s = src[sel].astype(np.int64)
        d = (dst[sel] - p * Nl).astype(np.int64)
        blk = d >> 7
        st = s // st_rows
        order = np.lexsort((s, st, blk))
        percore.append((s[order], d[order], blk[order], st[order]))

    counts = np.zeros((N_CORES, NB, NST), np.int64)
    for p in range(N_CORES):
        _, _, blk, st = percore[p]
        np.add.at(counts, (p, blk, st), 1)
    nchunks = (counts.max(axis=0) + 127) // 128          # [NB, NST]

    # canonical chunk emission order
    chunk_meta = []   # (isb, st, b) per chunk
    calls = []        # (st, chunk_lo, n_chunks)
    for isb in range(NSB):
        blocks = range(isb * SB, min((isb + 1) * SB, NB))
        for st in range(NST):
            run_lo = len(chunk_meta)
            for b in blocks:
                for _ in range(int(nchunks[b, st])):
                    chunk_meta.append((isb, st, b))
            n = len(chunk_meta) - run_lo
            o = run_lo
            while n > 0:
                take = min(n, MAX_CALL)
                calls.append((st, o, take))
                o += take
                n -= take
    NCH = len(chunk_meta)

    # first/last chunk index per (isb, b) for PSUM start/stop flags
    first = {}
    last = {}
    for ci, (isb, st, b) in enumerate(chunk_meta):
        key = (isb, b)
        if key not in first:
            first[key] = ci
        last[key] = ci

    # per-core edge arrays in padded chunk order
    idx_all = np.zeros((N_CORES, NCH * 128), np.int16)
    doff_all = np.full((N_CORES, NCH * 128), 255.0, np.float32)
    for p in range(N_CORES):
        s, d, blk, st = percore[p]
        # build run boundaries of the (blk, st)-sorted edge list
        runs = {}
        i = 0
        M = len(s)
        while i < M:
            k = (blk[i], st[i])
            j = i
            while j < M and blk[j] == k[0] and st[j] == k[1]:
                j += 1
            runs[k] = (i, j)
            i = j
        cursor = {k: v[0] for k, v in runs.items()}
        for ci, (isb, t, b) in enumerate(chunk_meta):
            base = ci * 128
            k = (b, t)
            if k in runs:
                lo = cursor[k]
                hi = min(lo + 128, runs[k][1])
                n = hi - lo
                cursor[k] = hi
                if n > 0:
                    idx_all[p, base:base + n] = (s[lo:hi] - t * st_rows).astype(np.int16)
                    doff_all[p, base:base + n] = (d[lo:hi] - b * 128).astype(np.float32)
        for k, (lo, hi) in runs.items():
            assert cursor[k] == hi, "edge run not fully consumed"

    # gather-call wrapped idx layout: per call [16, n/16], concat on free axis
    idxw_cols = NCH * 8
    idx_wrapped = np.zeros((N_CORES, 16, idxw_cols), np.int16)
    col = 0
    call_cols = []
    for (t, lo, nch) in calls:
        n = nch * 128
        for p in range(N_CORES):
            seg = idx_all[p, lo * 128: lo * 128 + n]
            idx_wrapped[p, :, col:col + n // 16] = seg.reshape(-1, 16).T
        call_cols.append(col)
        col += n // 16
    assert col == idxw_cols

    # dstoff duplicated pairs [128, 2*NCH] so the one-hot build's broadcast
    # has a unit-stride last dim (DVE 4x mode): doff2[p, 2c+j] = doff(c, p)
    doff = doff_all.reshape(N_CORES, NCH, 128).transpose(0, 2, 1)  # [p,128,NCH]
    doff2 = np.repeat(doff, 2, axis=2)                             # [p,128,2NCH]
    # dstrep [128, 4*NCH]: dstrep[p, 4c+j] = doff_edge(c, 32j + p%32)
    j_idx = np.arange(4)
    p_idx = np.arange(128)
    e_idx = (32 * j_idx[None, :] + (p_idx % 32)[:, None])      # [128, 4]
    dstrep = np.empty((N_CORES, 128, 4 * NCH), np.float32)
    for p in range(N_CORES):
        d3 = doff_all[p].reshape(NCH, 128)                      # [NCH, 128e]
        rep = d3[:, e_idx]                                      # [NCH, 128, 4]
        dstrep[p] = rep.transpose(1, 0, 2).reshape(128, NCH * 4)
    groups = []
    for (t, lo, nch) in calls:
        g = lo
        while g < lo + nch:
            take = min(GS, lo + nch - g)
            groups.append((t, lo, g, take))  # (st, call_lo, group_lo, size)
            g += take

    return dict(Nl=Nl, NB=NB, NSB=NSB, NST=NST, st_rows=st_rows, NCH=NCH,
                chunk_meta=chunk_meta, calls=calls, call_cols=call_cols,
                groups=groups, first=first, last=last,
                idx_wrapped=idx_wrapped, doff2=doff2, dstrep=dstrep,
                idxw_cols=idxw_cols)



def _balance_partition(src, dst, N):
    """Quartile-preserving node re-partition: pack nodes into (core, block)
    bins so per-(block, subtable) edge counts stay under 128-multiple quotas
    (minimizes gather-chunk padding and equalizes cores). Returns perm with
    perm[orig_id] = new_id; new id stays inside the node's src-subtable."""
    Nl = N // N_CORES
    NB = (Nl + 127) // 128
    NST = max(1, math.ceil(N / ST_MAX_ROWS))
    st_rows = math.ceil(N / NST)
    if st_rows % Nl != 0:
        return np.arange(N, dtype=np.int64)
    CPQ = st_rows // Nl
    NBINS = CPQ * NB
    st_arr = src // st_rows
    deg4 = np.zeros((N, NST), np.int32)
    np.add.at(deg4, (dst, st_arr), 1)

    perm = np.empty(N, np.int64)
    for q in range(NST):
        nodes = np.arange(q * st_rows, min((q + 1) * st_rows, N))
        d4 = deg4[nodes].astype(np.float64)
        order = np.argsort(-d4.sum(1), kind="stable")
        cap_n = np.full(NBINS, 128, np.int64)
        for c in range(CPQ):
            cap_n[c * NB + NB - 1] = Nl - 128 * (NB - 1)
        # per-(core, st) quotas in multiples of 128
        cap_e = np.zeros((NBINS, NST))
        tot_s = d4.sum(0) / CPQ                      # per-core totals
        for c in range(CPQ):
            sl = slice(c * NB, (c + 1) * NB)
            qb = tot_s[None, :] * (cap_n[sl, None] / cap_n[sl].sum())
            base = np.floor(qb / 128).astype(np.int64)
            caps = base.copy()
            for s in range(NST):
                need = int(math.ceil(
                    (tot_s[s] * 1.01 + 256 - 128 * base[:, s].sum()) / 128))
                if need > 0:
                    o = np.argsort(-(qb[:, s] / 128 - base[:, s]))
                    for i in range(need):
                        caps[o[i % NB], s] += 1
            cap_e[sl] = caps * 128.0
        sums = np.zeros((NBINS, NST))
        cnt = np.zeros(NBINS, np.int64)
        assign = np.empty(len(nodes), np.int64)
        for i in order:
            v = d4[i]
            ns = sums + v
            feasible = (ns <= cap_e).all(axis=1) & (cnt < cap_n)
            if feasible.any():
                score = (ns / np.maximum(cap_e, 1)).max(axis=1)
                score[~feasible] = np.inf
                b = int(np.argmin(score))
            else:
                over = np.maximum(ns - cap_e, 0).sum(axis=1)
                over[cnt >= cap_n] = np.inf
                b = int(np.argmin(over))
            assign[i] = b
            sums[b] += v
            cnt[b] += 1
        for _ in range(60):
            bad = np.argwhere(sums > cap_e)
            if len(bad) == 0:
                break
            fixed = 0
            for b, s in bad:
                while sums[b, s] > cap_e[b, s]:
                    members = np.where(assign == b)[0]
                    need = sums[b, s] - cap_e[b, s]
                    md = d4[members, s]
                    okm = members[md >= need]
                    mi = okm[np.argmin(d4[okm, s])] if len(okm) else \
                        members[np.argmax(md)]
                    vout = d4[mi]
                    slack = cap_e - sums
                    ok = (slack[:, s] >= vout[s])
                    ok[b] = False
                    if not ok.any():
                        break
                    done = False
                    cb = np.nonzero(ok)[0]
                    for tb in cb[np.argsort(-slack[cb, s])][:16]:
                        tmem = np.where(assign == tb)[0]
                        vd = d4[tmem]
                        newA = sums[b] - vout + vd
                        newB = sums[tb] + vout - vd
                        okv = (newA <= cap_e[b]).all(axis=1) & \
                            (newB <= cap_e[tb]).all(axis=1)
                        if not okv.any():
                            continue
                        cand = tmem[okv]
                        vi = cand[np.argmin(d4[cand, s])]
                        assign[mi], assign[vi] = tb, b
                        sums[b] += d4[vi] - vout
                        sums[tb] += vout - d4[vi]
                        fixed += 1
                        done = True
                        break
                    if not done:
                        break
            if fixed == 0:
                break
        for b in range(NBINS):
            members = nodes[assign == np.int64(b)]
            core = q * CPQ + b // NB
            blk = b % NB
            base = core * Nl + 128 * blk
            perm[members] = base + np.arange(len(members))
    return perm


# ----------------------------------------------------------------------------
# device program
# ----------------------------------------------------------------------------

def _split_multi_waits(nc):
    """walrus codegen only accepts one sync-wait per instruction; hoist any
    extra waits onto same-engine NOPs inserted right before the instruction."""
    n_id = 0
    for f in nc.m.functions:
        for blk in f.blocks:
            out = []
            for ins in blk.instructions:
                si = ins.sync_info
                if si is not None and len(si.on_wait) > 1 \
                        and ins.engine is not None:
                    waits = list(si.on_wait)
                    for w in waits[:-1]:
                        nop = mybir.InstNoOp(name=f"I-wsplit-{n_id}", ins=[],
                                             outs=[])
                        n_id += 1
                        nop.engine = ins.engine
                        nop.sync_info = mybir.SyncInfo(on_wait=[w],
                                                       on_update=[])
                        nc.inst_map[nop.name] = nop
                        out.append(nop)
                    ins.sync_info = mybir.SyncInfo(on_wait=[waits[-1]],
                                                   on_update=list(si.on_update))
                out.append(ins)
            blk.instructions = out

def _ap(base, *dims):
    """Rebuild AP with the same tensor/offset/partition dim, custom free dims."""
    return bass.AP(base.tensor, base.offset,
                   [list(base.ap[0])] + [list(d) for d in dims])


def _build(meta, N, D, H):
    Nl, NB, NSB, NST = meta["Nl"], meta["NB"], meta["NSB"], meta["NST"]
    st_rows = meta["st_rows"]
    NBP = NB * 128
    HD = H * D            # 256
    TW = HD               # table row: g head-minor, 512B bf16
    RW = HD + H           # 260: [exp | weighted msg] accumulate width

    nc = bass.Bass("TRN2", target_bir_lowering=False, debug=False,
                   enable_asserts=False, num_devices=N_CORES,
                   num_swdge_queues=NSWQ)

    # ---- DRAM tensors
    xT_in = nc.dram_tensor("xT_in", [D, N], BF16, kind="ExternalInput")
    xTl_in = nc.dram_tensor("xTl_in", [D, NBP], BF16, kind="ExternalInput")
    xc_in = nc.dram_tensor("xc_in", [128, NB, 2, D], F32, kind="ExternalInput")
    waug_in = nc.dram_tensor("waug_in", [D, TW], BF16, kind="ExternalInput")
    wr_in = nc.dram_tensor("wr_in", [D, H], BF16, kind="ExternalInput")
    mbig_in = nc.dram_tensor("mbig_in", [128, 2, D], BF16, kind="ExternalInput")
    iota_in = nc.dram_tensor("iota_in", [128, 128], BF16, kind="ExternalInput")
    itld_in = nc.dram_tensor("itld_in", [128, 32], BF16, kind="ExternalInput")
    ident_in = nc.dram_tensor("ident_in", [128, 128], BF16, kind="ExternalInput")
    scal_in = nc.dram_tensor("scal_in", [128, 4], F32, kind="ExternalInput")
    idx_in = nc.dram_tensor("idx_in", [128, meta["idxw_cols"]], I16,
                            kind="ExternalInput")
    doff2_in = nc.dram_tensor("doff2_in", [128, 2 * meta["NCH"]], BF16,
                              kind="ExternalInput")
    drep_in = nc.dram_tensor("drep_in", [128, 4 * meta["NCH"]], BF16,
                             kind="ExternalInput")

    # one table tensor per src-subtable so gathers of subtable q only wait
    # on that quarter's projection writes (projection/gather overlap)
    split_tbl = (NST > 1 and st_rows % Nl == 0)
    ntbl = NST if split_tbl else 1
    tables = [nc.dram_tensor(f"table{q}",
                             [min(st_rows, N - q * st_rows) if split_tbl else N,
                              TW], BF16, kind="Internal")
              for q in range(ntbl)]
    xp_mid = nc.dram_tensor("xp_mid", [128, NB, D], F32, kind="Internal")
    xT_sh = nc.dram_tensor("xT_sh", [D, NBP], BF16, kind="Internal")
    xT_ag = nc.dram_tensor("xT_ag", [D * N_CORES, NBP], BF16, kind="Internal",
                           addr_space="Shared")
    x_out = nc.dram_tensor("x_out", [Nl, D], F32, kind="ExternalOutput")

    from contextlib import ExitStack
    with tile.TileContext(nc) as tc, ExitStack() as es_:
        nc.gpsimd.load_library(library_config.mlp)
        cp = es_.enter_context(tc.tile_pool(name="consts", bufs=1))
        pools = {}
        for nm, bufs in [("xt", 4), ("rows", ROWS_BUFS), ("mask", 2),
                         ("rhs", 3),
                         ("sm", 3), ("tbl", 4), ("blk", 3), ("big", 2)]:
            pools[nm] = es_.enter_context(tc.tile_pool(name=nm, bufs=bufs))
        pA = es_.enter_context(tc.tile_pool(name="pacc", bufs=1, space="PSUM"))
        pB = es_.enter_context(tc.tile_pool(name="per8", bufs=2, space="PSUM"))
        pC = es_.enter_context(tc.tile_pool(name="ppj", bufs=2, space="PSUM"))

        # ---- load constants
        iota_t = cp.tile([128, 128], BF16, tag="iota")
        itld_t = cp.tile([128, 32], BF16, tag="itld")
        ident_t = cp.tile([128, 128], BF16, tag="ident")
        waug_t = cp.tile([D, TW], BF16, tag="waug")
        wr_t = cp.tile([D, H], BF16, tag="wr")
        mbig_t = cp.tile([128, 2, D], BF16, tag="mbig")
        scal_t = cp.tile([128, 4], F32, tag="scal")
        idx_t = cp.tile([128, meta["idxw_cols"]], I16, tag="idx")
        doff2_t = cp.tile([128, 2 * meta["NCH"]], BF16, tag="doff2")
        drep_t = cp.tile([128, 4 * meta["NCH"]], BF16, tag="drep")
        for t, s in [(iota_t, iota_in), (itld_t, itld_in), (ident_t, ident_in),
                     (waug_t, waug_in), (wr_t, wr_in), (mbig_t, mbig_in),
                     (scal_t, scal_in), (idx_t, idx_in), (doff2_t, doff2_in),
                     (drep_t, drep_in)]:
            nc.sync.dma_start(t[:], s.ap()[:])

        tails = {NB - 1: Nl - 128 * (NB - 1)}
        nidx_regs = {}

        def nidx_reg(n):
            if n not in nidx_regs:
                nidx_regs[n] = nc.gpsimd.to_reg(n)
            return nidx_regs[n]

        for step in range(STEP):
            # ------------------------------------------------ projection
            # emitted lazily per src-subtable so gathers overlap projection
            eng_flip = 0
            PB = 8
            proj_done = [False] * N_CORES

            def emit_proj_r(r, step=step):
                nonlocal eng_flip
                for t0 in range(0, NB, PB):
                    bts = list(range(t0, min(t0 + PB, NB)))
                    o = 128 * t0
                    w = min(128 * PB, Nl - o)
                    g0 = r * Nl + o
                    xt = pools["xt"].tile([D, 128 * PB], BF16, tag="projlhs")
                    ldeng = getattr(nc, LOAD_ENG)
                    if step == 0:
                        ldeng.dma_start(xt[:, :w], xT_in.ap()[:, g0:g0 + w])
                    else:
                        ldeng.dma_start(
                            xt[:, :w], xT_ag.ap()[D * r:D * (r + 1), o:o + w])
                    tb4 = pools["tbl"].tile([128, PB, TW], BF16, tag="tbl")
                    q = (g0 // st_rows) if split_tbl else 0
                    tq = tables[q]
                    gq = g0 - q * st_rows if split_tbl else g0
                    nfull = 0
                    part = None
                    for j, t in enumerate(bts):
                        wj = min(128, Nl - 128 * t)
                        scr = pC.tile([128, 512], F32, tag="scr")
                        nc.tensor.matmul(scr[:wj, 0:TW],
                                         xt[:, 128 * j:128 * j + wj],
                                         waug_t[:], start=True, stop=True)
                        if eng_flip % 2 == 0:
                            nc.vector.tensor_copy(tb4[:wj, j, :],
                                                  scr[:wj, 0:TW])
                        else:
                            nc.scalar.activation(tb4[:wj, j, :],
                                                 scr[:wj, 0:TW], AF.Copy)
                        eng_flip += 1
                        if wj == 128:
                            nfull += 1
                        else:
                            part = (j, wj)
                    if nfull:
                        dst = bass.AP(tq.ap().tensor, gq * TW,
                                      [[TW, 128], [128 * TW, nfull], [1, TW]])
                        nc.sync.dma_start(dst, tb4[:, :nfull, :])
                    if part is not None:
                        j, wj = part
                        gp = gq + 128 * j
                        nc.sync.dma_start(tq.ap()[gp:gp + wj, :],
                                          tb4[:wj, j, :])

            CPQ_T = st_rows // Nl if split_tbl else N_CORES
            LAZY = bool(int(os.environ.get("GAT_LAZYPROJ", "0")))
            if not LAZY:
                for r in range(N_CORES):
                    proj_done[r] = True
                    emit_proj_r(r)

            def ensure_proj(stq):
                if not LAZY:
                    return
                rs = range(CPQ_T * stq, CPQ_T * (stq + 1)) if split_tbl \
                    else range(N_CORES)
                for r in rs:
                    if not proj_done[r]:
                        proj_done[r] = True
                        emit_proj_r(r)

            # ------------------------------------------------ gather + attn
            xt_src = xTl_in if step == 0 else xT_sh
            call_i = 0
            group_i = 0
            for isb in range(NSB):
                blocks = list(range(isb * SB, min((isb + 1) * SB, NB)))
                nb = len(blocks)
                b0 = blocks[0]
                acc = pA.tile([128, SB, 512], F32, tag="acc")
                # x4p = (1-alpha)*x + alpha*lamda*x0 — precombined for step>0
                x4p = pools["blk"].tile([128, SB, D], F32, tag="x4p")
                ldeng = getattr(nc, LOAD_ENG)
                if step == 0:
                    xc4 = pools["blk"].tile([128, SB, 2, D], F32, tag="xc4")
                    ldeng.dma_start(xc4[:, :nb, :, :],
                                    xc_in.ap()[:, b0:b0 + nb, :, :])
                    nc.vector.scalar_tensor_tensor(
                        x4p[:, :nb, :], xc4[:, :nb, 0, :], scal_t[:, 0:1],
                        xc4[:, :nb, 1, :], op0=ALU.mult, op1=ALU.add)
                else:
                    ldeng.dma_start(x4p[:, :nb, :],
                                    xp_mid.ap()[:, b0:b0 + nb, :])
                # er for the superblock: one [D, SB*128] load + per-block matmul
                wsb = min(SB * 128, NBP - 128 * b0)
                xtb4 = pools["xt"].tile([D, SB * 128], BF16, tag="erlhs")
                ldeng.dma_start(
                    xtb4[:, :wsb], xt_src.ap()[:, 128 * b0:128 * b0 + wsb])
                es4 = pools["sm"].tile([128, SB, H], BF16, tag="er4")
                for j, b in enumerate(blocks):
                    nc.tensor.matmul(acc[:, j, 264:264 + H],
                                     xtb4[:, 128 * j:128 * (j + 1)], wr_t[:],
                                     start=True, stop=True)
                    nc.scalar.activation(es4[:, j, :], acc[:, j, 264:264 + H],
                                         AF.Copy)

                # walk this superblock's calls/groups/chunks
                while call_i < len(meta["calls"]):
                    st, lo, nch = meta["calls"][call_i]
                    if lo >= len(meta["chunk_meta"]) or \
                       meta["chunk_meta"][lo][0] != isb:
                        break
                    ensure_proj(st)
                    n = nch * 128
                    rows = pools["rows"].tile([128, MAX_CALL, TW], BF16,
                                              tag="rows")
                    icol = meta["call_cols"][call_i]
                    rows_ap = _ap(rows[:], [TW, nch], [1, TW])
                    if split_tbl:
                        tbl_ap = tables[st].ap()[:, :]
                    else:
                        tbl_ap = tables[0].ap()[st * st_rows:
                                                min((st + 1) * st_rows, N), :]
                    if not SKIP_GATHER:
                        nc.gpsimd.dma_gather(
                            rows_ap, tbl_ap, idx_t[:, icol:icol + n // 16],
                            num_idxs=n, num_idxs_reg=nidx_reg(n), elem_size=TW,
                            single_packet=bool(int(os.environ.get(
                                "GAT_SP1", "0"))),
                            queue_num=call_i % NSWQ)
                    call_i += 1

                    while group_i < len(meta["groups"]):
                        gst, glo_call, g, gs = meta["groups"][group_i]
                        if glo_call != lo:
                            break
                        group_i += 1
                        cc0 = g - lo   # chunk offset within call
                        # one-hot dst mask [128e, gs, 128n] — 2x TT form
                        # (doff2 pair-duplication keeps last dim unit-stride)
                        m8 = pools["mask"].tile([128, GS, 128], BF16, tag="m8")
                        nc.vector.tensor_tensor(
                            _ap(m8[:], [128, gs], [2, 64], [1, 2]),
                            _ap(iota_t[:], [0, gs], [2, 64], [1, 2]),
                            _ap(doff2_t[:, 2 * g:2 * (g + gs)],
                                [2, gs], [0, 64], [1, 2]),
                            op=ALU.is_equal)
                        mT8 = pools["mask"].tile([128, GS, 128], BF16, tag="mT8")
                        if MT8_MODE == "dve":
                            # one is_equal (pre-arranged drep layout) + one
                            # 32x32-block stream transpose per group
                            mt8 = pools["mask"].tile([128, GS, 128], BF16,
                                                     tag="mt8")
                            nc.vector.tensor_tensor(
                                _ap(mt8[:], [128, gs], [1, 128]),
                                _ap(drep_t[:, 4 * g:4 * (g + gs)],
                                    [4, gs], [1, 4], [0, 32]),
                                _ap(itld_t[:], [0, gs], [0, 4], [1, 32]),
                                op=ALU.is_equal)
                            nc.vector.transpose(
                                _ap(mT8[:], [1, gs * 128]),
                                _ap(mt8[:], [1, gs * 128]))
                        else:
                            # per-chunk PE transposes into PSUM (8-chunk
                            # batches), then batched copies (alt DVE/Act)
                            for h0 in range(0, gs, 8):
                                hn = min(8, gs - h0)
                                ms = pB.tile([128, 512], F32, tag="scr8")
                                for k in range(hn):
                                    nc.tensor.transpose(
                                        ms[:, 64 * k:64 * (k + 1)].bitcast(BF16),
                                        m8[:, h0 + k, :], ident_t[:])
                                if (group_i + h0) % 2 == 0:
                                    nc.vector.tensor_copy(
                                        _ap(mT8[:, h0:h0 + hn, :],
                                            [1, hn * 128]),
                                        ms[:, 0:64 * hn].bitcast(BF16))
                                else:
                                    nc.scalar.activation(
                                        _ap(mT8[:, h0:h0 + hn, :],
                                            [1, hn * 128]),
                                        ms[:, 0:64 * hn].bitcast(BF16),
                                        AF.Copy)
                        er8 = pB.tile([128, 512], F32, tag="scr8")
                        for k in range(gs):
                            ci = g + k
                            _, _, b = meta["chunk_meta"][ci]
                            j = b - b0
                            nc.tensor.matmul(er8[:, H * k:H * (k + 1)],
                                             mT8[:, k, :], es4[:, j, :],
                                             start=True, stop=True)
                        # t8 = er[dst] + el[src] (el = first H cols of rows)
                        t8 = pools["sm"].tile([128, GS * H], BF16, tag="t8")
                        nc.vector.tensor_tensor(
                            _ap(t8[:], [H, gs], [1, H]),
                            _ap(er8[:], [H, gs], [1, H]),
                            _ap(rows[:, cc0:cc0 + gs, 0:H], [TW, gs], [1, H]),
                            op=ALU.add)
                        lr8 = pools["sm"].tile([128, GS * H], BF16, tag="lr8")
                        nc.vector.scalar_tensor_tensor(
                            lr8[:, :gs * H], t8[:, :gs * H], scal_t[:, 3:4],
                            t8[:, :gs * H], op0=ALU.mult, op1=ALU.max)
                        rhs8 = pools["rhs"].tile([128, GS, RW], BF16, tag="rhs8")
                        nc.scalar.activation(
                            _ap(rhs8[:], [RW, gs], [1, H]),
                            _ap(lr8[:], [H, gs], [1, H]), AF.Exp)
                        # msg = g_row * exp — 2x TT form (head-minor layout
                        # keeps the exp broadcast's last dim unit-stride)
                        nc.vector.tensor_tensor(
                            _ap(rhs8[:, :, H:RW], [RW, gs], [H, D], [1, H]),
                            _ap(rows[:, cc0:cc0 + gs, :],
                                [TW, gs], [H, D], [1, H]),
                            _ap(rhs8[:], [RW, gs], [0, D], [1, H]),
                            op=ALU.mult)
                        for k in range(gs):
                            ci = g + k
                            _, _, b = meta["chunk_meta"][ci]
                            j = b - b0
                            nc.tensor.matmul(
                                acc[:, j, 0:RW], m8[:, k, :], rhs8[:, k, :],
                                start=(meta["first"][(isb, b)] == ci),
                                stop=(meta["last"][(isb, b)] == ci),
                                skip_group_check=True)

                # ---- superblock epilogue (batched over blocks)
                smax = pools["sm"].tile([128, SB * H], F32, tag="smax")
                nc.vector.tensor_scalar(
                    _ap(smax[:], [H, nb], [1, H]),
                    _ap(acc[:], [512, nb], [1, H]),
                    1e-30, None, op0=ALU.max)
                srec = pools["sm"].tile([128, SB * H], F32, tag="srec")
                nc.vector.reciprocal(srec[:, :nb * H], smax[:, :nb * H])
                srec2 = pools["sm"].tile([128, SB * H], F32, tag="srec2")
                nc.vector.tensor_scalar(
                    srec2[:, :nb * H], srec[:, :nb * H], scal_t[:, 1:2], None,
                    op0=ALU.mult)
                # normalized messages, bf16, head-minor [n, (d,h)]
                onorm = pools["big"].tile([128, SB, HD], BF16, tag="onorm")
                nc.vector.tensor_tensor(
                    _ap(onorm[:], [HD, nb], [H, D], [1, H]),
                    _ap(acc[:, :, H:RW], [512, nb], [H, D], [1, H]),
                    _ap(srec2[:], [H, nb], [0, D], [1, H]),
                    op=ALU.mult)
                xn = pools["blk"].tile([128, SB, D], F32, tag="xn")
                for j, b in enumerate(blocks):
                    # un-rotate + head-mean: out = onorm_j @ Mbig  (256->64)
                    scr = pC.tile([128, 512], F32, tag="scr")
                    nc.tensor.transpose(scr[:, 256:320].bitcast(BF16),
                                        onorm[:, j, 0:128], ident_t[:])
                    nc.tensor.transpose(scr[:, 320:384].bitcast(BF16),
                                        onorm[:, j, 128:256], ident_t[:])
                    oTs = pools["big"].tile([128, 2, 128], BF16, tag="oTs")
                    nc.scalar.activation(_ap(oTs[:], [1, 256]),
                                         scr[:, 256:384].bitcast(BF16),
                                         AF.Copy)
                    red = pB.tile([128, 512], F32, tag="scr8")
                    nc.tensor.matmul(red[:, 0:D], oTs[:, 0, :], mbig_t[:, 0, :],
                                     start=True, stop=False)
                    nc.tensor.matmul(red[:, 0:D], oTs[:, 1, :], mbig_t[:, 1, :],
                                     start=False, stop=True)
                    nc.vector.tensor_add(xn[:, j, :], x4p[:, j, :],
                                         red[:, 0:D])
                if step < STEP - 1:
                    if step > 0:
                        xc4 = pools["blk"].tile([128, SB, 2, D], F32,
                                                tag="xc4")
                        nc.sync.dma_start(
                            xc4[:, :nb, 1, :],
                            xc_in.ap()[:, b0:b0 + nb, 1, :])
                    xp = pools["blk"].tile([128, SB, D], F32, tag="xp")
                    nc.vector.scalar_tensor_tensor(
                        xp[:, :nb, :], xn[:, :nb, :], scal_t[:, 0:1],
                        xc4[:, :nb, 1, :], op0=ALU.mult, op1=ALU.add)
                    nc.sync.dma_start(xp_mid.ap()[:, b0:b0 + nb, :],
                                      xp[:, :nb, :])
                    xnb = pools["blk"].tile([128, SB, D], BF16, tag="xnb")
                    nc.vector.tensor_copy(xnb[:, :nb, :], xn[:, :nb, :])
                    xts4 = pools["sm"].tile([D, SB, 128], BF16, tag="xts")
                    for j, b in enumerate(blocks):
                        scr = pC.tile([128, 512], F32, tag="scr")
                        nc.tensor.transpose(scr[:D, 0:64].bitcast(BF16),
                                            xnb[:, j, :], ident_t[:])
                        nc.scalar.activation(xts4[:, j, :],
                                             scr[:D, 0:64].bitcast(BF16),
                                             AF.Copy)
                    nc.sync.dma_start(
                        xT_sh.ap()[:, 128 * b0:128 * (b0 + nb)],
                        xts4[:, :nb, :])
                else:
                    for j, b in enumerate(blocks):
                        w = tails.get(b, 128)
                        nc.sync.dma_start(x_out.ap()[128 * b:128 * b + w, :],
                                          xn[:w, j, :])
            assert call_i == len(meta["calls"]) and \
                group_i == len(meta["groups"])
            for r in range(N_CORES):
                if not proj_done[r]:
                    proj_done[r] = True
                    emit_proj_r(r)

            if step == 0 and STEP > 1 and not SKIP_COLL:
                nc.gpsimd.collective_compute(
                    "AllGather", ALU.bypass,
                    replica_groups=[list(range(N_CORES))],
                    ins=[xT_sh.ap()[:]], outs=[xT_ag.ap()[:]])

    _split_multi_waits(nc)
    lower_extended_insts(nc)
    return nc


# ----------------------------------------------------------------------------
# entry point
# ----------------------------------------------------------------------------

def _fold_weights(W, attn_l, D, H):
    """Per-head change of basis T_h = D_h @ Householder_h with g = T_h h,
    g[0] = attn_l[h]·h exactly. Returns (waug head-minor [D, H*D],
    Mbig [H*D, D] un-rotation, head-minor rows)."""
    W3 = W.reshape(D, H, D).astype(np.float64)
    waug_hm = np.empty((D, H, D))   # [k, h, d] -> col (d*H + h)
    mbig = np.empty((H, D, D))
    for h in range(H):
        a = attn_l[h].astype(np.float64)
        norm = np.linalg.norm(a)
        v = a / norm
        sign = 1.0 if v[0] >= 0 else -1.0
        u = v.copy()
        u[0] += sign
        u /= np.linalg.norm(u)
        House = np.eye(D) - 2.0 * np.outer(u, u)   # maps v -> -sign*e0
        T = House.copy()
        T[0, :] *= -sign * norm                    # D_h @ House: g[0] = a·h
        waug_hm[:, h, :] = W3[:, h, :] @ T.T
        Minv = House.copy()
        Minv[0, :] *= 1.0 / (-sign * norm)         # Mbig = Dinv @ House:
        mbig[h] = Minv                             # out[n,j] = Σ_k g[n,k]·Mbig[k,j]
    # head-minor interleave: waug[:, d*H + h]
    waug = waug_hm.transpose(0, 2, 1).reshape(D, H * D)
    mbig_hm = mbig.transpose(1, 0, 2).reshape(H * D, D)  # row (d*H+h) -> [D]
    return waug, mbig_hm


def kernel(x, x0, src, dst, W, attn_l, attn_r, alpha, lamda, **kw):
    global _last_results
    x = np.asarray(x, np.float32)
    x0 = np.asarray(x0, np.float32)
    src = np.asarray(src)
    dst = np.asarray(dst)
    W = np.asarray(W, np.float32)
    attn_l = np.asarray(attn_l, np.float32)
    attn_r = np.asarray(attn_r, np.float32)
    alpha_f = float(np.asarray(alpha))
    lamda_f = float(np.asarray(lamda))

    N, D = x.shape
    H = attn_l.shape[0]
    assert N % N_CORES == 0
    if bool(int(os.environ.get("GAT_BALANCE", "1"))):
        perm = _balance_partition(src, dst, N)
    else:
        perm = np.arange(N, dtype=np.int64)
    inv = np.argsort(perm)
    x = x[inv]
    x0 = x0[inv]
    src = perm[src]
    dst = perm[dst]
    meta = _plan_and_arrays(src, dst, N)
    Nl, NB = meta["Nl"], meta["NB"]
    NBP = NB * 128

    nc = _build(meta, N, D, H)

    # host-side weight prep
    waug_f, mbig_f = _fold_weights(W, attn_l, D, H)
    W3 = W.reshape(D, H, D)
    WR = np.einsum("khd,hd->kh", W3, attn_r)
    waug = _bf(waug_f)
    wr = _bf(WR)
    mbig = _bf(mbig_f.reshape(2, 128, D).transpose(1, 0, 2))  # [128, 2, D]
    iota = _bf(np.tile(np.arange(128, dtype=np.float32)[None, :], (128, 1)))
    itld = _bf((32 * (np.arange(128)[:, None] // 32)
                + np.arange(32)[None, :]).astype(np.float32))
    ident = _bf(np.eye(128, dtype=np.float32))
    scal = np.zeros((128, 4), np.float32)
    scal[:, 0] = 1.0 - alpha_f
    scal[:, 1] = alpha_f / H
    scal[:, 3] = NEG_SLOPE
    c0 = (alpha_f * lamda_f) * x0

    xT = _bf(x.T).copy()                      # [D, N]
    in_maps = []
    for p in range(N_CORES):
        lo = p * Nl
        xl = np.zeros((NBP, D), np.float32)
        xl[:Nl] = x[lo:lo + Nl]
        c0l = np.zeros((NBP, D), np.float32)
        c0l[:Nl] = c0[lo:lo + Nl]
        xc = np.stack([xl.reshape(NB, 128, D).transpose(1, 0, 2),
                       c0l.reshape(NB, 128, D).transpose(1, 0, 2)],
                      axis=2)                 # [128, NB, 2, D]
        idxw = meta["idx_wrapped"][p]
        if os.environ.get("GAT_PROBE", "") == "sortidx":
            # timing-only probe: sort each call's indices (breaks results)
            idxw = idxw.copy()
            for ci, (t, lo, nch) in enumerate(meta["calls"]):
                cc = meta["call_cols"][ci]
                n = nch * 128
                seg = idxw[:, cc:cc + n // 16]
                flat = np.sort(seg.T.reshape(-1))
                idxw[:, cc:cc + n // 16] = flat.reshape(-1, 16).T
        in_maps.append({
            "xT_in": np.ascontiguousarray(xT),
            "xTl_in": np.ascontiguousarray(_bf(xl.T)),
            "xc_in": np.ascontiguousarray(xc),
            "waug_in": waug, "wr_in": wr, "mbig_in": mbig,
            "iota_in": iota, "itld_in": itld, "ident_in": ident,
            "scal_in": scal,
            "idx_in": np.ascontiguousarray(np.tile(idxw, (8, 1))),
            "doff2_in": np.ascontiguousarray(_bf(meta["doff2"][p])),
            "drep_in": np.ascontiguousarray(_bf(meta["dstrep"][p])),
        })

    global _last_nc, _last_in_maps
    _last_nc = nc
    _last_in_maps = in_maps
    if os.environ.get("GAT_EXEC", "") == "sim":
        from concourse.bass_interp import MultiCoreSim
        sim = MultiCoreSim(nc, N_CORES, num_workers=N_CORES)
        for p in range(N_CORES):
            for name, arr in in_maps[p].items():
                sim.cores[p].tensor(name)[:] = arr
        sim.simulate()
        out = np.concatenate([np.asarray(sim.cores[p].tensor("x_out"))
                              for p in range(N_CORES)], axis=0)
        return out[perm].astype(np.float32)
    trace = bool(int(os.environ.get("GAT_TRACE", "0")))
    res = run_bass_kernel_spmd(nc, in_maps, core_ids=list(range(N_CORES)),
                               trace=trace,
                               trace_cores=list(range(N_CORES)) if trace else None,
                               stitch_traces=False)
    _last_results = res
    out = np.concatenate([res.results[p]["x_out"] for p in range(N_CORES)],
                         axis=0)
    return out[perm].astype(np.float32)


# revision 39
# speedup vs baseline: 2.2094x; 1.0664x over previous
"""GAT (graph attention) message-passing kernel for Trainium2, 8 NeuronCores.

Strategy (graph/data parallel, dst-sharded):
  - Nodes are partitioned across 8 cores by destination id (12500 each).
  - Edges are sharded by dst partition, sorted by (dst-block, src-subtable),
    and padded so every core runs an identical (SPMD) program.
  - The attention left-term el is folded into the projection by a per-head
    Householder change of basis: g = T_h h with g[...,0] = el, stored
    head-minor so table rows are exactly [g interleaved (d,h)] = 512B bf16.
    The epilogue un-rotates with a 256x64 matmul per dst block.
  - Per step, every core projects ALL nodes into its HBM row table, then
    indirect-gathers g[src] rows per edge (dma_gather), builds one-hot dst
    masks on DVE (4x tensor-scalar forms where layouts allow), computes
    attention scores (er via a small maskT matmul), and accumulates
    [softmax-denominator | weighted message sum] into per-dst-block PSUM
    with mask matmuls on TensorE.
  - Block epilogue: normalize by the segment sum, un-rotate + head-mean via
    TensorE, residual update.
  - Between the 2 conv steps, the updated x (transposed, bf16) is AllGathered
    across the 8 cores.
"""

import os
import math
import numpy as np
import ml_dtypes

import concourse.bass as bass
import concourse.tile as tile
import concourse.mybir as mybir
from concourse import library_config
from concourse.library_overlay import lower_extended_insts
from concourse.bass_utils import run_bass_kernel_spmd

BF16 = mybir.dt.bfloat16
F32 = mybir.dt.float32
I16 = mybir.dt.int16
AF = mybir.ActivationFunctionType
ALU = mybir.AluOpType

NEG_SLOPE = 0.2
STEP = int(os.environ.get("GAT_STEPS", "2"))
SKIP_COLL = bool(int(os.environ.get("GAT_SKIP_COLL", "0")))
SKIP_GATHER = bool(int(os.environ.get("GAT_SKIP_GATHER", "0")))
N_CORES = 8
SB = 4            # blocks per superblock (PSUM accumulators alive at once)
MAX_CALL = int(os.environ.get("GAT_MAXCALL", "16"))
GS = int(os.environ.get("GAT_GS", "16"))
ROWS_BUFS = int(os.environ.get("GAT_ROWSBUFS", "6"))
LOAD_ENG = os.environ.get("GAT_LOADENG", "sync")
MT8_MODE = os.environ.get("GAT_MT8", "pe")
NSWQ = int(os.environ.get("GAT_NSWQ", "4"))
ST_MAX_ROWS = 25000   # subtable rows (int16 gather index limit)

_last_results = None  # BassKernelResults stash for test harness
_last_nc = None       # built Bass module (for test-side benching)
_last_in_maps = None  # per-core input maps (for test-side benching)


def _bf(x):
    return np.asarray(x, np.float32).astype(ml_dtypes.bfloat16)


# ----------------------------------------------------------------------------
# host-side preprocessing
# ----------------------------------------------------------------------------

def _plan_and_arrays(src, dst, N):
    """Shard/sort/pad edges; build the shared chunk plan and per-core arrays."""
    Nl = N // N_CORES
    NB = (Nl + 127) // 128
    NSB = (NB + SB - 1) // SB
    NST = max(1, math.ceil(N / ST_MAX_ROWS))
    st_rows = math.ceil(N / NST)

    core = dst // Nl
    percore = []
    for p in range(N_CORES):
        sel = np.nonzero(core == p)[0]
        s = src[sel].astype(np.int64)
        d = (dst[sel] - p * Nl).astype(np.int64)
        blk = d >> 7
        st = s // st_rows
        order = np.lexsort((s, st, blk))
        percore.append((s[order], d[order], blk[order], st[order]))

    counts = np.zeros((N_CORES, NB, NST), np.int64)
    for p in range(N_CORES):
        _, _, blk, st = percore[p]
        np.add.at(counts, (p, blk, st), 1)
    nchunks = (counts.max(axis=0) + 127) // 128          # [NB, NST]

    # canonical chunk emission order
    chunk_meta = []   # (isb, st, b) per chunk
    calls = []        # (st, chunk_lo, n_chunks)
    for isb in range(NSB):
        blocks = range(isb * SB, min((isb + 1) * SB, NB))
        for st in range(NST):
            run_lo = len(chunk_meta)
            for b in blocks:
                for _ in range(int(nchunks[b, st])):
                    chunk_meta.append((isb, st, b))
            n = len(chunk_meta) - run_lo
            o = run_lo
            while n > 0:
                take = min(n, MAX_CALL)
                calls.append((st, o, take))
                o += take
                n -= take
    NCH = len(chunk_meta)

    # first/last chunk index per (isb, b) for PSUM start/stop flags
    first = {}
    last = {}
    for ci, (isb, st, b) in enumerate(chunk_meta):
        key = (isb, b)
        if key not in first:
            first[key] = ci
        last[key] = ci

    # per-core edge arrays in padded chunk order
    idx_all = np.zeros((N_CORES, NCH * 128), np.int16)
    doff_all = np.full((N_CORES, NCH * 128), 255.0, np.float32)
    for p in range(N_CORES):
        s, d, blk, st = percore[p]
        # build run boundaries of the (blk, st)-sorted edge list
        runs = {}
        i = 0
        M = len(s)
        while i < M:
            k = (blk[i], st[i])
            j = i
            while j < M and blk[j] == k[0] and st[j] == k[1]:
                j += 1
            runs[k] = (i, j)
            i = j
        cursor = {k: v[0] for k, v in runs.items()}
        for ci, (isb, t, b) in enumerate(chunk_meta):
            base = ci * 128
            k = (b, t)
            if k in runs:
                lo = cursor[k]
                hi = min(lo + 128, runs[k][1])
                n = hi - lo
                cursor[k] = hi
                if n > 0:
                    idx_all[p, base:base + n] = (s[lo:hi] - t * st_rows).astype(np.int16)
                    doff_all[p, base:base + n] = (d[lo:hi] - b * 128).astype(np.float32)
        for k, (lo, hi) in runs.items():
            assert cursor[k] == hi, "edge run not fully consumed"

    # gather-call wrapped idx layout: per call [16, n/16], concat on free axis
    idxw_cols = NCH * 8
    idx_wrapped = np.zeros((N_CORES, 16, idxw_cols), np.int16)
    col = 0
    call_cols = []
    for (t, lo, nch) in calls:
        n = nch * 128
        for p in range(N_CORES):
            seg = idx_all[p, lo * 128: lo * 128 + n]
            idx_wrapped[p, :, col:col + n // 16] = seg.reshape(-1, 16).T
        call_cols.append(col)
        col += n // 16
    assert col == idxw_cols

    # dstoff duplicated pairs [128, 2*NCH] so the one-hot build's broadcast
    # has a unit-stride last dim (DVE 4x mode): doff2[p, 2c+j] = doff(c, p)
    doff = doff_all.reshape(N_CORES, NCH, 128).transpose(0, 2, 1)  # [p,128,NCH]
    doff2 = np.repeat(doff, 2, axis=2)                             # [p,128,2NCH]
    # dstrep [128, 4*NCH]: dstrep[p, 4c+j] = doff_edge(c, 32j + p%32)
    j_idx = np.arange(4)
    p_idx = np.arange(128)
    e_idx = (32 * j_idx[None, :] + (p_idx % 32)[:, None])      # [128, 4]
    dstrep = np.empty((N_CORES, 128, 4 * NCH), np.float32)
    for p in range(N_CORES):
        d3 = doff_all[p].reshape(NCH, 128)                      # [NCH, 128e]
        rep = d3[:, e_idx]                                      # [NCH, 128, 4]
        dstrep[p] = rep.transpose(1, 0, 2).reshape(128, NCH * 4)
    groups = []
    for (t, lo, nch) in calls:
        g = lo
        while g < lo + nch:
            take = min(GS, lo + nch - g)
            groups.append((t, lo, g, take))  # (st, call_lo, group_lo, size)
            g += take

    return dict(Nl=Nl, NB=NB, NSB=NSB, NST=NST, st_rows=st_rows, NCH=NCH,
                chunk_meta=chunk_meta, calls=calls, call_cols=call_cols,
                groups=groups, first=first, last=last,
                idx_wrapped=idx_wrapped, doff2=doff2, dstrep=dstrep,
                idxw_cols=idxw_cols)



def _balance_partition(src, dst, N):
    """Quartile-preserving node re-partition: pack nodes into (core, block)
    bins so per-(block, subtable) edge counts stay under 128-multiple quotas
    (minimizes gather-chunk padding and equalizes cores). Returns perm with
    perm[orig_id] = new_id; new id stays inside the node's src-subtable."""
    Nl = N // N_CORES
    NB = (Nl + 127) // 128
    NST = max(1, math.ceil(N / ST_MAX_ROWS))
    st_rows = math.ceil(N / NST)
    if st_rows % Nl != 0:
        return np.arange(N, dtype=np.int64)
    CPQ = st_rows // Nl
    NBINS = CPQ * NB
    st_arr = src // st_rows
    deg4 = np.zeros((N, NST), np.int32)
    np.add.at(deg4, (dst, st_arr), 1)

    perm = np.empty(N, np.int64)
    for q in range(NST):
        nodes = np.arange(q * st_rows, min((q + 1) * st_rows, N))
        d4 = deg4[nodes].astype(np.float64)
        order = np.argsort(-d4.sum(1), kind="stable")
        cap_n = np.full(NBINS, 128, np.int64)
        for c in range(CPQ):
            cap_n[c * NB + NB - 1] = Nl - 128 * (NB - 1)
        # per-(core, st) quotas in multiples of 128
        cap_e = np.zeros((NBINS, NST))
        tot_s = d4.sum(0) / CPQ                      # per-core totals
        for c in range(CPQ):
            sl = slice(c * NB, (c + 1) * NB)
            qb = tot_s[None, :] * (cap_n[sl, None] / cap_n[sl].sum())
            base = np.floor(qb / 128).astype(np.int64)
            caps = base.copy()
            for s in range(NST):
                need = int(math.ceil(
                    (tot_s[s] * 1.01 + 256 - 128 * base[:, s].sum()) / 128))
                if need > 0:
                    o = np.argsort(-(qb[:, s] / 128 - base[:, s]))
                    for i in range(need):
                        caps[o[i % NB], s] += 1
            cap_e[sl] = caps * 128.0
        sums = np.zeros((NBINS, NST))
        cnt = np.zeros(NBINS, np.int64)
        assign = np.empty(len(nodes), np.int64)
        for i in order:
            v = d4[i]
            ns = sums + v
            feasible = (ns <= cap_e).all(axis=1) & (cnt < cap_n)
            if feasible.any():
                score = (ns / np.maximum(cap_e, 1)).max(axis=1)
                score[~feasible] = np.inf
                b = int(np.argmin(score))
            else:
                over = np.maximum(ns - cap_e, 0).sum(axis=1)
                over[cnt >= cap_n] = np.inf
                b = int(np.argmin(over))
            assign[i] = b
            sums[b] += v
            cnt[b] += 1
        for _ in range(60):
            bad = np.argwhere(sums > cap_e)
            if len(bad) == 0:
                break
            fixed = 0
            for b, s in bad:
                while sums[b, s] > cap_e[b, s]:
                    members = np.where(assign == b)[0]
                    need = sums[b, s] - cap_e[b, s]
                    md = d4[members, s]
                    okm = members[md >= need]
                    mi = okm[np.argmin(d4[okm, s])] if len(okm) else \
                        members[np.argmax(md)]
                    vout = d4[mi]
                    slack = cap_e - sums
                    ok = (slack[:, s] >= vout[s])
                    ok[b] = False
                    if not ok.any():
                        break
                    done = False
                    cb = np.nonzero(ok)[0]
                    for tb in cb[np.argsort(-slack[cb, s])][:16]:
                        tmem = np.where(assign == tb)[0]
                        vd = d4[tmem]
                        newA = sums[b] - vout + vd
                        newB = sums[tb] + vout - vd
                        okv = (newA <= cap_e[b]).all(axis=1) & \
                            (newB <= cap_e[tb]).all(axis=1)
                        if not okv.any():
                            continue
                        cand = tmem[okv]
                        vi = cand[np.argmin(d4[cand, s])]
                        assign[mi], assign[vi] = tb, b
                        sums[b] += d4[vi] - vout
                        sums[tb] += vout - d4[vi]
                        fixed += 1
                        done = True
                        break
                    if not done:
                        break
            if fixed == 0:
                break
        for b in range(NBINS):
            members = nodes[assign == np.int64(b)]
            core = q * CPQ + b // NB
            blk = b % NB
            base = core * Nl + 128 * blk
            perm[members] = base + np.arange(len(members))
    return perm


# ----------------------------------------------------------------------------
# device program
# ----------------------------------------------------------------------------

def _split_multi_waits(nc):
    """walrus codegen only accepts one sync-wait per instruction; hoist any
    extra waits onto same-engine NOPs inserted right before the instruction."""
    n_id = 0
    for f in nc.m.functions:
        for blk in f.blocks:
            out = []
            for ins in blk.instructions:
                si = ins.sync_info
                if si is not None and len(si.on_wait) > 1 \
                        and ins.engine is not None:
                    waits = list(si.on_wait)
                    for w in waits[:-1]:
                        nop = mybir.InstNoOp(name=f"I-wsplit-{n_id}", ins=[],
                                             outs=[])
                        n_id += 1
                        nop.engine = ins.engine
                        nop.sync_info = mybir.SyncInfo(on_wait=[w],
                                                       on_update=[])
                        nc.inst_map[nop.name] = nop
                        out.append(nop)
                    ins.sync_info = mybir.SyncInfo(on_wait=[waits[-1]],
                                                   on_update=list(si.on_update))
                out.append(ins)
            blk.instructions = out

def _ap(base, *dims):
    """Rebuild AP with the same tensor/offset/partition dim, custom free dims."""
    return bass.AP(base.tensor, base.offset,
                   [list(base.ap[0])] + [list(d) for d in dims])


def _build(meta, N, D, H):
    Nl, NB, NSB, NST = meta["Nl"], meta["NB"], meta["NSB"], meta["NST"]
    st_rows = meta["st_rows"]
    NBP = NB * 128
    HD = H * D            # 256
    TW = HD               # table row: g head-minor, 512B bf16
    RW = HD + H           # 260: [exp | weighted msg] accumulate width

    nc = bass.Bass("TRN2", target_bir_lowering=False, debug=False,
                   enable_asserts=False, num_devices=N_CORES,
                   num_swdge_queues=NSWQ)

    # ---- DRAM tensors
    xT_in = nc.dram_tensor("xT_in", [D, N], BF16, kind="ExternalInput")
    xTl_in = nc.dram_tensor("xTl_in", [D, NBP], BF16, kind="ExternalInput")
    xc_in = nc.dram_tensor("xc_in", [128, NB, 2, D], F32, kind="ExternalInput")
    waug_in = nc.dram_tensor("waug_in", [D, TW], BF16, kind="ExternalInput")
    wr_in = nc.dram_tensor("wr_in", [D, H], BF16, kind="ExternalInput")
    mbig_in = nc.dram_tensor("mbig_in", [128, 2, D], BF16, kind="ExternalInput")
    iota_in = nc.dram_tensor("iota_in", [128, 128], BF16, kind="ExternalInput")
    itld_in = nc.dram_tensor("itld_in", [128, 32], BF16, kind="ExternalInput")
    ident_in = nc.dram_tensor("ident_in", [128, 128], BF16, kind="ExternalInput")
    scal_in = nc.dram_tensor("scal_in", [128, 4], F32, kind="ExternalInput")
    idx_in = nc.dram_tensor("idx_in", [128, meta["idxw_cols"]], I16,
                            kind="ExternalInput")
    doff2_in = nc.dram_tensor("doff2_in", [128, 2 * meta["NCH"]], BF16,
                              kind="ExternalInput")
    drep_in = nc.dram_tensor("drep_in", [128, 4 * meta["NCH"]], BF16,
                             kind="ExternalInput")

    # one table tensor per src-subtable so gathers of subtable q only wait
    # on that quarter's projection writes (projection/gather overlap)
    split_tbl = (NST > 1 and st_rows % Nl == 0)
    ntbl = NST if split_tbl else 1
    tables = [nc.dram_tensor(f"table{q}",
                             [min(st_rows, N - q * st_rows) if split_tbl else N,
                              TW], BF16, kind="Internal")
              for q in range(ntbl)]
    xp_mid = nc.dram_tensor("xp_mid", [128, NB, D], F32, kind="Internal")
    xT_sh = nc.dram_tensor("xT_sh", [D, NBP], BF16, kind="Internal")
    xT_ag = nc.dram_tensor("xT_ag", [D * N_CORES, NBP], BF16, kind="Internal",
                           addr_space="Shared")
    x_out = nc.dram_tensor("x_out", [Nl, D], F32, kind="ExternalOutput")

    from contextlib import ExitStack
    with tile.TileContext(nc) as tc, ExitStack() as es_:
        nc.gpsimd.load_library(library_config.mlp)
        cp = es_.enter_context(tc.tile_pool(name="consts", bufs=1))
        pools = {}
        MB_ = int(os.environ.get("GAT_MASKBUFS", "2"))
        RB_ = int(os.environ.get("GAT_RHSBUFS", "3"))
        for nm, bufs in [("xt", 4), ("rows", ROWS_BUFS), ("mask", MB_),
                         ("rhs", RB_),
                         ("sm", 3), ("tbl", 4), ("blk", 3), ("big", 2)]:
            pools[nm] = es_.enter_context(tc.tile_pool(name=nm, bufs=bufs))
        pA = es_.enter_context(tc.tile_pool(name="pacc", bufs=1, space="PSUM"))
        pB = es_.enter_context(tc.tile_pool(name="per8", bufs=2, space="PSUM"))
        pC = es_.enter_context(tc.tile_pool(name="ppj", bufs=2, space="PSUM"))

        # ---- load constants
        iota_t = cp.tile([128, 128], BF16, tag="iota")
        itld_t = cp.tile([128, 32], BF16, tag="itld")
        ident_t = cp.tile([128, 128], BF16, tag="ident")
        waug_t = cp.tile([D, TW], BF16, tag="waug")
        wr_t = cp.tile([D, H], BF16, tag="wr")
        mbig_t = cp.tile([128, 2, D], BF16, tag="mbig")
        scal_t = cp.tile([128, 4], F32, tag="scal")
        idx_t = cp.tile([128, meta["idxw_cols"]], I16, tag="idx")
        doff2_t = cp.tile([128, 2 * meta["NCH"]], BF16, tag="doff2")
        drep_t = cp.tile([128, 4 * meta["NCH"]], BF16, tag="drep")
        for t, s in [(iota_t, iota_in), (itld_t, itld_in), (ident_t, ident_in),
                     (waug_t, waug_in), (wr_t, wr_in), (mbig_t, mbig_in),
                     (scal_t, scal_in), (idx_t, idx_in), (doff2_t, doff2_in),
                     (drep_t, drep_in)]:
            nc.sync.dma_start(t[:], s.ap()[:])

        tails = {NB - 1: Nl - 128 * (NB - 1)}
        nidx_regs = {}

        def nidx_reg(n):
            if n not in nidx_regs:
                nidx_regs[n] = nc.gpsimd.to_reg(n)
            return nidx_regs[n]

        for step in range(STEP):
            # ------------------------------------------------ projection
            # emitted lazily per src-subtable so gathers overlap projection
            eng_flip = 0
            PB = 8
            proj_done = [False] * N_CORES

            def emit_proj_r(r, step=step):
                nonlocal eng_flip
                for t0 in range(0, NB, PB):
                    bts = list(range(t0, min(t0 + PB, NB)))
                    o = 128 * t0
                    w = min(128 * PB, Nl - o)
                    g0 = r * Nl + o
                    xt = pools["xt"].tile([D, 128 * PB], BF16, tag="projlhs")
                    ldeng = getattr(nc, LOAD_ENG)
                    if step == 0:
                        ldeng.dma_start(xt[:, :w], xT_in.ap()[:, g0:g0 + w])
                    else:
                        ldeng.dma_start(
                            xt[:, :w], xT_ag.ap()[D * r:D * (r + 1), o:o + w])
                    tb4 = pools["tbl"].tile([128, PB, TW], BF16, tag="tbl")
                    q = (g0 // st_rows) if split_tbl else 0
                    tq = tables[q]
                    gq = g0 - q * st_rows if split_tbl else g0
                    nfull = 0
                    part = None
                    for j, t in enumerate(bts):
                        wj = min(128, Nl - 128 * t)
                        scr = pC.tile([128, 512], F32, tag="scr")
                        nc.tensor.matmul(scr[:wj, 0:TW],
                                         xt[:, 128 * j:128 * j + wj],
                                         waug_t[:], start=True, stop=True)
                        if eng_flip % 2 == 0:
                            nc.vector.tensor_copy(tb4[:wj, j, :],
                                                  scr[:wj, 0:TW])
                        else:
                            nc.scalar.activation(tb4[:wj, j, :],
                                                 scr[:wj, 0:TW], AF.Copy)
                        eng_flip += 1
                        if wj == 128:
                            nfull += 1
                        else:
                            part = (j, wj)
                    if nfull:
                        dst = bass.AP(tq.ap().tensor, gq * TW,
                                      [[TW, 128], [128 * TW, nfull], [1, TW]])
                        nc.sync.dma_start(dst, tb4[:, :nfull, :])
                    if part is not None:
                        j, wj = part
                        gp = gq + 128 * j
                        nc.sync.dma_start(tq.ap()[gp:gp + wj, :],
                                          tb4[:wj, j, :])

            CPQ_T = st_rows // Nl if split_tbl else N_CORES
            LAZY = bool(int(os.environ.get("GAT_LAZYPROJ", "0")))
            if not LAZY:
                for r in range(N_CORES):
                    proj_done[r] = True
                    emit_proj_r(r)

            def ensure_proj(stq):
                if not LAZY:
                    return
                rs = range(CPQ_T * stq, CPQ_T * (stq + 1)) if split_tbl \
                    else range(N_CORES)
                for r in rs:
                    if not proj_done[r]:
                        proj_done[r] = True
                        emit_proj_r(r)

            # ------------------------------------------------ gather + attn
            xt_src = xTl_in if step == 0 else xT_sh
            call_i = 0
            group_i = 0
            for isb in range(NSB):
                blocks = list(range(isb * SB, min((isb + 1) * SB, NB)))
                nb = len(blocks)
                b0 = blocks[0]
                acc = pA.tile([128, SB, 512], F32, tag="acc")
                # x4p = (1-alpha)*x + alpha*lamda*x0 — precombined for step>0
                x4p = pools["blk"].tile([128, SB, D], F32, tag="x4p")
                ldeng = getattr(nc, LOAD_ENG)
                if step == 0:
                    xc4 = pools["blk"].tile([128, SB, 2, D], F32, tag="xc4")
                    ldeng.dma_start(xc4[:, :nb, :, :],
                                    xc_in.ap()[:, b0:b0 + nb, :, :])
                    nc.vector.scalar_tensor_tensor(
                        x4p[:, :nb, :], xc4[:, :nb, 0, :], scal_t[:, 0:1],
                        xc4[:, :nb, 1, :], op0=ALU.mult, op1=ALU.add)
                else:
                    ldeng.dma_start(x4p[:, :nb, :],
                                    xp_mid.ap()[:, b0:b0 + nb, :])
                # er for the superblock: one [D, SB*128] load + per-block matmul
                wsb = min(SB * 128, NBP - 128 * b0)
                xtb4 = pools["xt"].tile([D, SB * 128], BF16, tag="erlhs")
                ldeng.dma_start(
                    xtb4[:, :wsb], xt_src.ap()[:, 128 * b0:128 * b0 + wsb])
                es4 = pools["sm"].tile([128, SB, H], BF16, tag="er4")
                for j, b in enumerate(blocks):
                    nc.tensor.matmul(acc[:, j, 264:264 + H],
                                     xtb4[:, 128 * j:128 * (j + 1)], wr_t[:],
                                     start=True, stop=True)
                    nc.scalar.activation(es4[:, j, :], acc[:, j, 264:264 + H],
                                         AF.Copy)

                # walk this superblock's calls/groups/chunks
                while call_i < len(meta["calls"]):
                    st, lo, nch = meta["calls"][call_i]
                    if lo >= len(meta["chunk_meta"]) or \
                       meta["chunk_meta"][lo][0] != isb:
                        break
                    ensure_proj(st)
                    n = nch * 128
                    rows = pools["rows"].tile([128, MAX_CALL, TW], BF16,
                                              tag="rows")
                    icol = meta["call_cols"][call_i]
                    rows_ap = _ap(rows[:], [TW, nch], [1, TW])
                    if split_tbl:
                        tbl_ap = tables[st].ap()[:, :]
                    else:
                        tbl_ap = tables[0].ap()[st * st_rows:
                                                min((st + 1) * st_rows, N), :]
                    if not SKIP_GATHER:
                        nc.gpsimd.dma_gather(
                            rows_ap, tbl_ap, idx_t[:, icol:icol + n // 16],
                            num_idxs=n, num_idxs_reg=nidx_reg(n), elem_size=TW,
                            single_packet=bool(int(os.environ.get(
                                "GAT_SP1", "0"))),
                            queue_num=call_i % NSWQ)
                    call_i += 1

                    while group_i < len(meta["groups"]):
                        gst, glo_call, g, gs = meta["groups"][group_i]
                        if glo_call != lo:
                            break
                        group_i += 1
                        cc0 = g - lo   # chunk offset within call
                        # one-hot dst mask [128e, gs, 128n] — 2x TT form
                        # (doff2 pair-duplication keeps last dim unit-stride)
                        m8 = pools["mask"].tile([128, GS, 128], BF16, tag="m8")
                        nc.vector.tensor_tensor(
                            _ap(m8[:], [128, gs], [2, 64], [1, 2]),
                            _ap(iota_t[:], [0, gs], [2, 64], [1, 2]),
                            _ap(doff2_t[:, 2 * g:2 * (g + gs)],
                                [2, gs], [0, 64], [1, 2]),
                            op=ALU.is_equal)
                        mT8 = pools["mask"].tile([128, GS, 128], BF16, tag="mT8")
                        if MT8_MODE == "dve":
                            # one is_equal (pre-arranged drep layout) + one
                            # 32x32-block stream transpose per group
                            mt8 = pools["mask"].tile([128, GS, 128], BF16,
                                                     tag="mt8")
                            nc.vector.tensor_tensor(
                                _ap(mt8[:], [128, gs], [1, 128]),
                                _ap(drep_t[:, 4 * g:4 * (g + gs)],
                                    [4, gs], [1, 4], [0, 32]),
                                _ap(itld_t[:], [0, gs], [0, 4], [1, 32]),
                                op=ALU.is_equal)
                            nc.vector.transpose(
                                _ap(mT8[:], [1, gs * 128]),
                                _ap(mt8[:], [1, gs * 128]))
                        else:
                            # per-chunk PE transposes into PSUM (8-chunk
                            # batches), then batched copies (alt DVE/Act)
                            for h0 in range(0, gs, 8):
                                hn = min(8, gs - h0)
                                ms = pB.tile([128, 512], F32, tag="scr8")
                                for k in range(hn):
                                    nc.tensor.transpose(
                                        ms[:, 64 * k:64 * (k + 1)].bitcast(BF16),
                                        m8[:, h0 + k, :], ident_t[:])
                                if (group_i + h0) % 2 == 0:
                                    nc.vector.tensor_copy(
                                        _ap(mT8[:, h0:h0 + hn, :],
                                            [1, hn * 128]),
                                        ms[:, 0:64 * hn].bitcast(BF16))
                                else:
                                    nc.scalar.activation(
                                        _ap(mT8[:, h0:h0 + hn, :],
                                            [1, hn * 128]),
                                        ms[:, 0:64 * hn].bitcast(BF16),
                                        AF.Copy)
                        er8 = pB.tile([128, 512], F32, tag="scr8")
                        for k in range(gs):
                            ci = g + k
                            _, _, b = meta["chunk_meta"][ci]
                            j = b - b0
                            nc.tensor.matmul(er8[:, H * k:H * (k + 1)],
                                             mT8[:, k, :], es4[:, j, :],
                                             start=True, stop=True)
                        # t8 = er[dst] + el[src] (el = first H cols of rows)
                        t8 = pools["sm"].tile([128, GS * H], BF16, tag="t8")
                        nc.vector.tensor_tensor(
                            _ap(t8[:], [H, gs], [1, H]),
                            _ap(er8[:], [H, gs], [1, H]),
                            _ap(rows[:, cc0:cc0 + gs, 0:H], [TW, gs], [1, H]),
                            op=ALU.add)
                        lr8 = pools["sm"].tile([128, GS * H], BF16, tag="lr8")
                        nc.vector.scalar_tensor_tensor(
                            lr8[:, :gs * H], t8[:, :gs * H], scal_t[:, 3:4],
                            t8[:, :gs * H], op0=ALU.mult, op1=ALU.max)
                        rhs8 = pools["rhs"].tile([128, GS, RW], BF16, tag="rhs8")
                        nc.scalar.activation(
                            _ap(rhs8[:], [RW, gs], [1, H]),
                            _ap(lr8[:], [H, gs], [1, H]), AF.Exp)
                        # msg = g_row * exp — 2x TT form (head-minor layout
                        # keeps the exp broadcast's last dim unit-stride)
                        nc.vector.tensor_tensor(
                            _ap(rhs8[:, :, H:RW], [RW, gs], [H, D], [1, H]),
                            _ap(rows[:, cc0:cc0 + gs, :],
                                [TW, gs], [H, D], [1, H]),
                            _ap(rhs8[:], [RW, gs], [0, D], [1, H]),
                            op=ALU.mult)
                        for k in range(gs):
                            ci = g + k
                            _, _, b = meta["chunk_meta"][ci]
                            j = b - b0
                            nc.tensor.matmul(
                                acc[:, j, 0:RW], m8[:, k, :], rhs8[:, k, :],
                                start=(meta["first"][(isb, b)] == ci),
                                stop=(meta["last"][(isb, b)] == ci),
                                skip_group_check=True)

                # ---- superblock epilogue (batched over blocks)
                smax = pools["sm"].tile([128, SB * H], F32, tag="smax")
                nc.vector.tensor_scalar(
                    _ap(smax[:], [H, nb], [1, H]),
                    _ap(acc[:], [512, nb], [1, H]),
                    1e-30, None, op0=ALU.max)
                srec = pools["sm"].tile([128, SB * H], F32, tag="srec")
                nc.vector.reciprocal(srec[:, :nb * H], smax[:, :nb * H])
                srec2 = pools["sm"].tile([128, SB * H], F32, tag="srec2")
                nc.vector.tensor_scalar(
                    srec2[:, :nb * H], srec[:, :nb * H], scal_t[:, 1:2], None,
                    op0=ALU.mult)
                # normalized messages, bf16, head-minor [n, (d,h)]
                onorm = pools["big"].tile([128, SB, HD], BF16, tag="onorm")
                nc.vector.tensor_tensor(
                    _ap(onorm[:], [HD, nb], [H, D], [1, H]),
                    _ap(acc[:, :, H:RW], [512, nb], [H, D], [1, H]),
                    _ap(srec2[:], [H, nb], [0, D], [1, H]),
                    op=ALU.mult)
                xn = pools["blk"].tile([128, SB, D], F32, tag="xn")
                for j, b in enumerate(blocks):
                    # un-rotate + head-mean: out = onorm_j @ Mbig  (256->64)
                    scr = pC.tile([128, 512], F32, tag="scr")
                    nc.tensor.transpose(scr[:, 256:320].bitcast(BF16),
                                        onorm[:, j, 0:128], ident_t[:])
                    nc.tensor.transpose(scr[:, 320:384].bitcast(BF16),
                                        onorm[:, j, 128:256], ident_t[:])
                    oTs = pools["big"].tile([128, 2, 128], BF16, tag="oTs")
                    nc.scalar.activation(_ap(oTs[:], [1, 256]),
                                         scr[:, 256:384].bitcast(BF16),
                                         AF.Copy)
                    red = pB.tile([128, 512], F32, tag="scr8")
                    nc.tensor.matmul(red[:, 0:D], oTs[:, 0, :], mbig_t[:, 0, :],
                                     start=True, stop=False)
                    nc.tensor.matmul(red[:, 0:D], oTs[:, 1, :], mbig_t[:, 1, :],
                                     start=False, stop=True)
                    nc.vector.tensor_add(xn[:, j, :], x4p[:, j, :],
                                         red[:, 0:D])
                if step < STEP - 1:
                    if step > 0:
                        xc4 = pools["blk"].tile([128, SB, 2, D], F32,
                                                tag="xc4")
                        nc.sync.dma_start(
                            xc4[:, :nb, 1, :],
                            xc_in.ap()[:, b0:b0 + nb, 1, :])
                    xp = pools["blk"].tile([128, SB, D], F32, tag="xp")
                    nc.vector.scalar_tensor_tensor(
                        xp[:, :nb, :], xn[:, :nb, :], scal_t[:, 0:1],
                        xc4[:, :nb, 1, :], op0=ALU.mult, op1=ALU.add)
                    nc.sync.dma_start(xp_mid.ap()[:, b0:b0 + nb, :],
                                      xp[:, :nb, :])
                    xnb = pools["blk"].tile([128, SB, D], BF16, tag="xnb")
                    nc.vector.tensor_copy(xnb[:, :nb, :], xn[:, :nb, :])
                    xts4 = pools["sm"].tile([D, SB, 128], BF16, tag="xts")
                    for j, b in enumerate(blocks):
                        scr = pC.tile([128, 512], F32, tag="scr")
                        nc.tensor.transpose(scr[:D, 0:64].bitcast(BF16),
                                            xnb[:, j, :], ident_t[:])
                        nc.scalar.activation(xts4[:, j, :],
                                             scr[:D, 0:64].bitcast(BF16),
                                             AF.Copy)
                    nc.sync.dma_start(
                        xT_sh.ap()[:, 128 * b0:128 * (b0 + nb)],
                        xts4[:, :nb, :])
                else:
                    for j, b in enumerate(blocks):
                        w = tails.get(b, 128)
                        nc.sync.dma_start(x_out.ap()[128 * b:128 * b + w, :],
                                          xn[:w, j, :])
            assert call_i == len(meta["calls"]) and \
                group_i == len(meta["groups"])
            for r in range(N_CORES):
                if not proj_done[r]:
                    proj_done[r] = True
                    emit_proj_r(r)

            if step == 0 and STEP > 1 and not SKIP_COLL:
                nc.gpsimd.collective_compute(
                    "AllGather", ALU.bypass,
                    replica_groups=[list(range(N_CORES))],
                    ins=[xT_sh.ap()[:]], outs=[xT_ag.ap()[:]])

    _split_multi_waits(nc)
    lower_extended_insts(nc)
    return nc


# ----------------------------------------------------------------------------
# entry point
# ----------------------------------------------------------------------------

def _fold_weights(W, attn_l, D, H):
    """Per-head change of basis T_h = D_h @ Householder_h with g = T_h h,
    g[0] = attn_l[h]·h exactly. Returns (waug head-minor [D, H*D],
    Mbig [H*D, D] un-rotation, head-minor rows)."""
    W3 = W.reshape(D, H, D).astype(np.float64)
    waug_hm = np.empty((D, H, D))   # [k, h, d] -> col (d*H + h)
    mbig = np.empty((H, D, D))
    for h in range(H):
        a = attn_l[h].astype(np.float64)
        norm = np.linalg.norm(a)
        v = a / norm
        sign = 1.0 if v[0] >= 0 else -1.0
        u = v.copy()
        u[0] += sign
        u /= np.linalg.norm(u)
        House = np.eye(D) - 2.0 * np.outer(u, u)   # maps v -> -sign*e0
        T = House.copy()
        T[0, :] *= -sign * norm                    # D_h @ House: g[0] = a·h
        waug_hm[:, h, :] = W3[:, h, :] @ T.T
        Minv = House.copy()
        Minv[0, :] *= 1.0 / (-sign * norm)         # Mbig = Dinv @ House:
        mbig[h] = Minv                             # out[n,j] = Σ_k g[n,k]·Mbig[k,j]
    # head-minor interleave: waug[:, d*H + h]
    waug = waug_hm.transpose(0, 2, 1).reshape(D, H * D)
    mbig_hm = mbig.transpose(1, 0, 2).reshape(H * D, D)  # row (d*H+h) -> [D]
    return waug, mbig_hm


def kernel(x, x0, src, dst, W, attn_l, attn_r, alpha, lamda, **kw):
    global _last_results
    x = np.asarray(x, np.float32)
    x0 = np.asarray(x0, np.float32)
    src = np.asarray(src)
    dst = np.asarray(dst)
    W = np.asarray(W, np.float32)
    attn_l = np.asarray(attn_l, np.float32)
    attn_r = np.asarray(attn_r, np.float32)
    alpha_f = float(np.asarray(alpha))
    lamda_f = float(np.asarray(lamda))

    N, D = x.shape
    H = attn_l.shape[0]
    assert N % N_CORES == 0
    if bool(int(os.environ.get("GAT_BALANCE", "1"))):
        perm = _balance_partition(src, dst, N)
    else:
        perm = np.arange(N, dtype=np.int64)
    inv = np.argsort(perm)
    x = x[inv]
    x0 = x0[inv]
    src = perm[src]
    dst = perm[dst]
    meta = _plan_and_arrays(src, dst, N)
    Nl, NB = meta["Nl"], meta["NB"]
    NBP = NB * 128

    nc = _build(meta, N, D, H)

    # host-side weight prep
    waug_f, mbig_f = _fold_weights(W, attn_l, D, H)
    W3 = W.reshape(D, H, D)
    WR = np.einsum("khd,hd->kh", W3, attn_r)
    waug = _bf(waug_f)
    wr = _bf(WR)
    mbig = _bf(mbig_f.reshape(2, 128, D).transpose(1, 0, 2))  # [128, 2, D]
    iota = _bf(np.tile(np.arange(128, dtype=np.float32)[None, :], (128, 1)))
    itld = _bf((32 * (np.arange(128)[:, None] // 32)
                + np.arange(32)[None, :]).astype(np.float32))
    ident = _bf(np.eye(128, dtype=np.float32))
    scal = np.zeros((128, 4), np.float32)
    scal[:, 0] = 1.0 - alpha_f
    scal[:, 1] = alpha_f / H
    scal[:, 3] = NEG_SLOPE
    c0 = (alpha_f * lamda_f) * x0

    xT = _bf(x.T).copy()                      # [D, N]
    in_maps = []
    for p in range(N_CORES):
        lo = p * Nl
        xl = np.zeros((NBP, D), np.float32)
        xl[:Nl] = x[lo:lo + Nl]
        c0l = np.zeros((NBP, D), np.float32)
        c0l[:Nl] = c0[lo:lo + Nl]
        xc = np.stack([xl.reshape(NB, 128, D).transpose(1, 0, 2),
                       c0l.reshape(NB, 128, D).transpose(1, 0, 2)],
                      axis=2)                 # [128, NB, 2, D]
        idxw = meta["idx_wrapped"][p]
        if os.environ.get("GAT_PROBE", "") == "sortidx":
            # timing-only probe: sort each call's indices (breaks results)
            idxw = idxw.copy()
            for ci, (t, lo, nch) in enumerate(meta["calls"]):
                cc = meta["call_cols"][ci]
                n = nch * 128
                seg = idxw[:, cc:cc + n // 16]
                flat = np.sort(seg.T.reshape(-1))
                idxw[:, cc:cc + n // 16] = flat.reshape(-1, 16).T
        in_maps.append({
            "xT_in": np.ascontiguousarray(xT),
            "xTl_in": np.ascontiguousarray(_bf(xl.T)),
            "xc_in": np.ascontiguousarray(xc),
            "waug_in": waug, "wr_in": wr, "mbig_in": mbig,
            "iota_in": iota, "itld_in": itld, "ident_in": ident,
            "scal_in": scal,
            "idx_in": np.ascontiguousarray(np.tile(idxw, (8, 1))),
            "doff2_in": np.ascontiguousarray(_bf(meta["doff2"][p])),
            "drep_in": np.ascontiguousarray(_bf(meta["dstrep"][p])),
        })

    global _last_nc, _last_in_maps
    _last_nc = nc
    _last_in_maps = in_maps
    if os.environ.get("GAT_EXEC", "") == "sim":
        from concourse.bass_interp import MultiCoreSim
        sim = MultiCoreSim(nc, N_CORES, num_workers=N_CORES)
        for p in range(N_CORES):
            for name, arr in in_maps[p].items():
                sim.cores[p].tensor(name)[:] = arr
        sim.simulate()
        out = np.concatenate([np.asarray(sim.cores[p].tensor("x_out"))
                              for p in range(N_CORES)], axis=0)
        return out[perm].astype(np.float32)
    trace = bool(int(os.environ.get("GAT_TRACE", "0")))
    res = run_bass_kernel_spmd(nc, in_maps, core_ids=list(range(N_CORES)),
                               trace=trace,
                               trace_cores=list(range(N_CORES)) if trace else None,
                               stitch_traces=False)
    _last_results = res
    out = np.concatenate([res.results[p]["x_out"] for p in range(N_CORES)],
                         axis=0)
    return out[perm].astype(np.float32)


# revision 41
# speedup vs baseline: 2.8575x; 1.2933x over previous
"""GAT (graph attention) message-passing kernel for Trainium2, 8 NeuronCores.

Strategy (graph/data parallel, dst-sharded):
  - Nodes are partitioned across 8 cores by destination id (12500 each).
  - Edges are sharded by dst partition, sorted by (dst-block, src-subtable),
    and padded so every core runs an identical (SPMD) program.
  - The attention left-term el is folded into the projection by a per-head
    Householder change of basis: g = T_h h with g[...,0] = el, stored
    head-minor so table rows are exactly [g interleaved (d,h)] = 512B bf16.
    The epilogue un-rotates with a 256x64 matmul per dst block.
  - Per step, every core projects ALL nodes into its HBM row table, then
    indirect-gathers g[src] rows per edge (dma_gather), builds one-hot dst
    masks on DVE (4x tensor-scalar forms where layouts allow), computes
    attention scores (er via a small maskT matmul), and accumulates
    [softmax-denominator | weighted message sum] into per-dst-block PSUM
    with mask matmuls on TensorE.
  - Block epilogue: normalize by the segment sum, un-rotate + head-mean via
    TensorE, residual update.
  - Between the 2 conv steps, the updated x (transposed, bf16) is AllGathered
    across the 8 cores.
"""

import os
import math
import numpy as np
import ml_dtypes

import concourse.bass as bass
import concourse.tile as tile
import concourse.mybir as mybir
from concourse import library_config
from concourse.library_overlay import lower_extended_insts
from concourse.bass_utils import run_bass_kernel_spmd

BF16 = mybir.dt.bfloat16
F32 = mybir.dt.float32
I16 = mybir.dt.int16
AF = mybir.ActivationFunctionType
ALU = mybir.AluOpType

NEG_SLOPE = 0.2
STEP = int(os.environ.get("GAT_STEPS", "2"))
SKIP_COLL = bool(int(os.environ.get("GAT_SKIP_COLL", "0")))
SKIP_GATHER = bool(int(os.environ.get("GAT_SKIP_GATHER", "0")))
N_CORES = 8
SB = 4            # blocks per superblock (PSUM accumulators alive at once)
MAX_CALL = int(os.environ.get("GAT_MAXCALL", "16"))
GS = int(os.environ.get("GAT_GS", "16"))
ROWS_BUFS = int(os.environ.get("GAT_ROWSBUFS", "6"))
LOAD_ENG = os.environ.get("GAT_LOADENG", "sync")
MT8_MODE = os.environ.get("GAT_MT8", "pe")
COPY_ENG = os.environ.get("GAT_COPYENG", "act")
NSWQ = int(os.environ.get("GAT_NSWQ", "4"))
ST_MAX_ROWS = 25000   # subtable rows (int16 gather index limit)

_last_results = None  # BassKernelResults stash for test harness
_last_nc = None       # built Bass module (for test-side benching)
_last_in_maps = None  # per-core input maps (for test-side benching)


def _bf(x):
    return np.asarray(x, np.float32).astype(ml_dtypes.bfloat16)


# ----------------------------------------------------------------------------
# host-side preprocessing
# ----------------------------------------------------------------------------

def _plan_and_arrays(src, dst, N):
    """Shard/sort/pad edges; build the shared chunk plan and per-core arrays."""
    Nl = N // N_CORES
    NB = (Nl + 127) // 128
    NSB = (NB + SB - 1) // SB
    NST = max(1, math.ceil(N / ST_MAX_ROWS))
    st_rows = math.ceil(N / NST)

    core = dst // Nl
    percore = []
    for p in range(N_CORES):
        sel = np.nonzero(core == p)[0]
        s = src[sel].astype(np.int64)
        d = (dst[sel] - p * Nl).astype(np.int64)
        blk = d >> 7
        st = s // st_rows
        order = np.lexsort((s, st, blk))
        percore.append((s[order], d[order], blk[order], st[order]))

    counts = np.zeros((N_CORES, NB, NST), np.int64)
    for p in range(N_CORES):
        _, _, blk, st = percore[p]
        np.add.at(counts, (p, blk, st), 1)
    nchunks = (counts.max(axis=0) + 127) // 128          # [NB, NST]

    # canonical chunk emission order
    chunk_meta = []   # (isb, st, b) per chunk
    calls = []        # (st, chunk_lo, n_chunks)
    for isb in range(NSB):
        blocks = range(isb * SB, min((isb + 1) * SB, NB))
        for st in range(NST):
            run_lo = len(chunk_meta)
            for b in blocks:
                for _ in range(int(nchunks[b, st])):
                    chunk_meta.append((isb, st, b))
            n = len(chunk_meta) - run_lo
            o = run_lo
            while n > 0:
                take = min(n, MAX_CALL)
                calls.append((st, o, take))
                o += take
                n -= take
    NCH = len(chunk_meta)

    # first/last chunk index per (isb, b) for PSUM start/stop flags
    first = {}
    last = {}
    for ci, (isb, st, b) in enumerate(chunk_meta):
        key = (isb, b)
        if key not in first:
            first[key] = ci
        last[key] = ci

    # per-core edge arrays in padded chunk order
    idx_all = np.zeros((N_CORES, NCH * 128), np.int16)
    doff_all = np.full((N_CORES, NCH * 128), 255.0, np.float32)
    for p in range(N_CORES):
        s, d, blk, st = percore[p]
        # build run boundaries of the (blk, st)-sorted edge list
        runs = {}
        i = 0
        M = len(s)
        while i < M:
            k = (blk[i], st[i])
            j = i
            while j < M and blk[j] == k[0] and st[j] == k[1]:
                j += 1
            runs[k] = (i, j)
            i = j
        cursor = {k: v[0] for k, v in runs.items()}
        for ci, (isb, t, b) in enumerate(chunk_meta):
            base = ci * 128
            k = (b, t)
            if k in runs:
                lo = cursor[k]
                hi = min(lo + 128, runs[k][1])
                n = hi - lo
                cursor[k] = hi
                if n > 0:
                    idx_all[p, base:base + n] = (s[lo:hi] - t * st_rows).astype(np.int16)
                    doff_all[p, base:base + n] = (d[lo:hi] - b * 128).astype(np.float32)
        for k, (lo, hi) in runs.items():
            assert cursor[k] == hi, "edge run not fully consumed"

    # gather-call wrapped idx layout: per call [16, n/16], concat on free axis
    idxw_cols = NCH * 8
    idx_wrapped = np.zeros((N_CORES, 16, idxw_cols), np.int16)
    col = 0
    call_cols = []
    for (t, lo, nch) in calls:
        n = nch * 128
        for p in range(N_CORES):
            seg = idx_all[p, lo * 128: lo * 128 + n]
            idx_wrapped[p, :, col:col + n // 16] = seg.reshape(-1, 16).T
        call_cols.append(col)
        col += n // 16
    assert col == idxw_cols

    # dstoff duplicated pairs [128, 2*NCH] so the one-hot build's broadcast
    # has a unit-stride last dim (DVE 4x mode): doff2[p, 2c+j] = doff(c, p)
    doff = doff_all.reshape(N_CORES, NCH, 128).transpose(0, 2, 1)  # [p,128,NCH]
    doff2 = np.repeat(doff, 2, axis=2)                             # [p,128,2NCH]
    # dstrep [128, 4*NCH]: dstrep[p, 4c+j] = doff_edge(c, 32j + p%32)
    j_idx = np.arange(4)
    p_idx = np.arange(128)
    e_idx = (32 * j_idx[None, :] + (p_idx % 32)[:, None])      # [128, 4]
    dstrep = np.empty((N_CORES, 128, 4 * NCH), np.float32)
    for p in range(N_CORES):
        d3 = doff_all[p].reshape(NCH, 128)                      # [NCH, 128e]
        rep = d3[:, e_idx]                                      # [NCH, 128, 4]
        dstrep[p] = rep.transpose(1, 0, 2).reshape(128, NCH * 4)
    groups = []
    for (t, lo, nch) in calls:
        g = lo
        while g < lo + nch:
            take = min(GS, lo + nch - g)
            groups.append((t, lo, g, take))  # (st, call_lo, group_lo, size)
            g += take

    return dict(Nl=Nl, NB=NB, NSB=NSB, NST=NST, st_rows=st_rows, NCH=NCH,
                chunk_meta=chunk_meta, calls=calls, call_cols=call_cols,
                groups=groups, first=first, last=last,
                idx_wrapped=idx_wrapped, doff2=doff2, dstrep=dstrep,
                idxw_cols=idxw_cols)



def _balance_partition(src, dst, N):
    """Quartile-preserving node re-partition: pack nodes into (core, block)
    bins so per-(block, subtable) edge counts stay under 128-multiple quotas
    (minimizes gather-chunk padding and equalizes cores). Returns perm with
    perm[orig_id] = new_id; new id stays inside the node's src-subtable."""
    Nl = N // N_CORES
    NB = (Nl + 127) // 128
    NST = max(1, math.ceil(N / ST_MAX_ROWS))
    st_rows = math.ceil(N / NST)
    if st_rows % Nl != 0:
        return np.arange(N, dtype=np.int64)
    CPQ = st_rows // Nl
    NBINS = CPQ * NB
    st_arr = src // st_rows
    deg4 = np.zeros((N, NST), np.int32)
    np.add.at(deg4, (dst, st_arr), 1)

    perm = np.empty(N, np.int64)
    for q in range(NST):
        nodes = np.arange(q * st_rows, min((q + 1) * st_rows, N))
        d4 = deg4[nodes].astype(np.float64)
        order = np.argsort(-d4.sum(1), kind="stable")
        cap_n = np.full(NBINS, 128, np.int64)
        for c in range(CPQ):
            cap_n[c * NB + NB - 1] = Nl - 128 * (NB - 1)
        # per-(core, st) quotas in multiples of 128
        cap_e = np.zeros((NBINS, NST))
        tot_s = d4.sum(0) / CPQ                      # per-core totals
        for c in range(CPQ):
            sl = slice(c * NB, (c + 1) * NB)
            qb = tot_s[None, :] * (cap_n[sl, None] / cap_n[sl].sum())
            base = np.floor(qb / 128).astype(np.int64)
            caps = base.copy()
            for s in range(NST):
                need = int(math.ceil(
                    (tot_s[s] * 1.01 + 256 - 128 * base[:, s].sum()) / 128))
                if need > 0:
                    o = np.argsort(-(qb[:, s] / 128 - base[:, s]))
                    for i in range(need):
                        caps[o[i % NB], s] += 1
            cap_e[sl] = caps * 128.0
        sums = np.zeros((NBINS, NST))
        cnt = np.zeros(NBINS, np.int64)
        assign = np.empty(len(nodes), np.int64)
        for i in order:
            v = d4[i]
            ns = sums + v
            feasible = (ns <= cap_e).all(axis=1) & (cnt < cap_n)
            if feasible.any():
                score = (ns / np.maximum(cap_e, 1)).max(axis=1)
                score[~feasible] = np.inf
                b = int(np.argmin(score))
            else:
                over = np.maximum(ns - cap_e, 0).sum(axis=1)
                over[cnt >= cap_n] = np.inf
                b = int(np.argmin(over))
            assign[i] = b
            sums[b] += v
            cnt[b] += 1
        for _ in range(60):
            bad = np.argwhere(sums > cap_e)
            if len(bad) == 0:
                break
            fixed = 0
            for b, s in bad:
                while sums[b, s] > cap_e[b, s]:
                    members = np.where(assign == b)[0]
                    need = sums[b, s] - cap_e[b, s]
                    md = d4[members, s]
                    okm = members[md >= need]
                    mi = okm[np.argmin(d4[okm, s])] if len(okm) else \
                        members[np.argmax(md)]
                    vout = d4[mi]
                    slack = cap_e - sums
                    ok = (slack[:, s] >= vout[s])
                    ok[b] = False
                    if not ok.any():
                        break
                    done = False
                    cb = np.nonzero(ok)[0]
                    for tb in cb[np.argsort(-slack[cb, s])][:16]:
                        tmem = np.where(assign == tb)[0]
                        vd = d4[tmem]
                        newA = sums[b] - vout + vd
                        newB = sums[tb] + vout - vd
                        okv = (newA <= cap_e[b]).all(axis=1) & \
                            (newB <= cap_e[tb]).all(axis=1)
                        if not okv.any():
                            continue
                        cand = tmem[okv]
                        vi = cand[np.argmin(d4[cand, s])]
                        assign[mi], assign[vi] = tb, b
                        sums[b] += d4[vi] - vout
                        sums[tb] += vout - d4[vi]
                        fixed += 1
                        done = True
                        break
                    if not done:
                        break
            if fixed == 0:
                break
        for b in range(NBINS):
            members = nodes[assign == np.int64(b)]
            core = q * CPQ + b // NB
            blk = b % NB
            base = core * Nl + 128 * blk
            perm[members] = base + np.arange(len(members))
    return perm


# ----------------------------------------------------------------------------
# device program
# ----------------------------------------------------------------------------

def _split_multi_waits(nc):
    """walrus codegen only accepts one sync-wait per instruction; hoist any
    extra waits onto same-engine NOPs inserted right before the instruction."""
    n_id = 0
    for f in nc.m.functions:
        for blk in f.blocks:
            out = []
            for ins in blk.instructions:
                si = ins.sync_info
                if si is not None and len(si.on_wait) > 1 \
                        and ins.engine is not None:
                    waits = list(si.on_wait)
                    for w in waits[:-1]:
                        nop = mybir.InstNoOp(name=f"I-wsplit-{n_id}", ins=[],
                                             outs=[])
                        n_id += 1
                        nop.engine = ins.engine
                        nop.sync_info = mybir.SyncInfo(on_wait=[w],
                                                       on_update=[])
                        nc.inst_map[nop.name] = nop
                        out.append(nop)
                    ins.sync_info = mybir.SyncInfo(on_wait=[waits[-1]],
                                                   on_update=list(si.on_update))
                out.append(ins)
            blk.instructions = out

def _ap(base, *dims):
    """Rebuild AP with the same tensor/offset/partition dim, custom free dims."""
    return bass.AP(base.tensor, base.offset,
                   [list(base.ap[0])] + [list(d) for d in dims])


def _build(meta, N, D, H):
    Nl, NB, NSB, NST = meta["Nl"], meta["NB"], meta["NSB"], meta["NST"]
    st_rows = meta["st_rows"]
    NBP = NB * 128
    HD = H * D            # 256
    TW = HD               # table row: g head-minor, 512B bf16
    RW = HD + H           # 260: [exp | weighted msg] accumulate width

    nc = bass.Bass("TRN2", target_bir_lowering=False, debug=False,
                   enable_asserts=False, num_devices=N_CORES,
                   num_swdge_queues=NSWQ)

    # ---- DRAM tensors
    xT_in = nc.dram_tensor("xT_in", [D, N], BF16, kind="ExternalInput")
    xTl_in = nc.dram_tensor("xTl_in", [D, NBP], BF16, kind="ExternalInput")
    xc_in = nc.dram_tensor("xc_in", [128, NB, 2, D], F32, kind="ExternalInput")
    waug_in = nc.dram_tensor("waug_in", [D, TW], BF16, kind="ExternalInput")
    wr_in = nc.dram_tensor("wr_in", [D, H], BF16, kind="ExternalInput")
    mbig_in = nc.dram_tensor("mbig_in", [128, 2, D], BF16, kind="ExternalInput")
    iota_in = nc.dram_tensor("iota_in", [128, 128], BF16, kind="ExternalInput")
    itld_in = nc.dram_tensor("itld_in", [128, 32], BF16, kind="ExternalInput")
    ident_in = nc.dram_tensor("ident_in", [128, 128], BF16, kind="ExternalInput")
    scal_in = nc.dram_tensor("scal_in", [128, 4], F32, kind="ExternalInput")
    idx_in = nc.dram_tensor("idx_in", [128, meta["idxw_cols"]], I16,
                            kind="ExternalInput")
    doff2_in = nc.dram_tensor("doff2_in", [128, 2 * meta["NCH"]], BF16,
                              kind="ExternalInput")
    drep_in = nc.dram_tensor("drep_in", [128, 4 * meta["NCH"]], BF16,
                             kind="ExternalInput")

    # one table tensor per src-subtable so gathers of subtable q only wait
    # on that quarter's projection writes (projection/gather overlap)
    split_tbl = (NST > 1 and st_rows % Nl == 0)
    ntbl = NST if split_tbl else 1
    tables = [nc.dram_tensor(f"table{q}",
                             [min(st_rows, N - q * st_rows) if split_tbl else N,
                              TW], BF16, kind="Internal")
              for q in range(ntbl)]
    xp_mid = nc.dram_tensor("xp_mid", [128, NB, D], F32, kind="Internal")
    xT_sh = nc.dram_tensor("xT_sh", [D, NBP], BF16, kind="Internal")
    xT_ag = nc.dram_tensor("xT_ag", [D * N_CORES, NBP], BF16, kind="Internal",
                           addr_space="Shared")
    x_out = nc.dram_tensor("x_out", [Nl, D], F32, kind="ExternalOutput")

    from contextlib import ExitStack
    with tile.TileContext(nc) as tc, ExitStack() as es_:
        nc.gpsimd.load_library(library_config.mlp)
        cp = es_.enter_context(tc.tile_pool(name="consts", bufs=1))
        pools = {}
        MB_ = int(os.environ.get("GAT_MASKBUFS", "2"))
        RB_ = int(os.environ.get("GAT_RHSBUFS", "3"))
        for nm, bufs in [("xt", 4), ("rows", ROWS_BUFS), ("mask", MB_),
                         ("rhs", RB_),
                         ("sm", 3), ("tbl", 4), ("blk", 3), ("big", 2)]:
            pools[nm] = es_.enter_context(tc.tile_pool(name=nm, bufs=bufs))
        pA = es_.enter_context(tc.tile_pool(name="pacc", bufs=1, space="PSUM"))
        pB = es_.enter_context(tc.tile_pool(name="per8", bufs=2, space="PSUM"))
        pC = es_.enter_context(tc.tile_pool(name="ppj", bufs=2, space="PSUM"))

        # ---- load constants
        iota_t = cp.tile([128, 128], BF16, tag="iota")
        itld_t = cp.tile([128, 32], BF16, tag="itld")
        ident_t = cp.tile([128, 128], BF16, tag="ident")
        waug_t = cp.tile([D, TW], BF16, tag="waug")
        wr_t = cp.tile([D, H], BF16, tag="wr")
        mbig_t = cp.tile([128, 2, D], BF16, tag="mbig")
        scal_t = cp.tile([128, 4], F32, tag="scal")
        idx_t = cp.tile([128, meta["idxw_cols"]], I16, tag="idx")
        doff2_t = cp.tile([128, 2 * meta["NCH"]], BF16, tag="doff2")
        drep_t = cp.tile([128, 4 * meta["NCH"]], BF16, tag="drep")
        for t, s in [(iota_t, iota_in), (itld_t, itld_in), (ident_t, ident_in),
                     (waug_t, waug_in), (wr_t, wr_in), (mbig_t, mbig_in),
                     (scal_t, scal_in), (idx_t, idx_in), (doff2_t, doff2_in),
                     (drep_t, drep_in)]:
            nc.sync.dma_start(t[:], s.ap()[:])

        tails = {NB - 1: Nl - 128 * (NB - 1)}
        nidx_regs = {}

        def nidx_reg(n):
            if n not in nidx_regs:
                nidx_regs[n] = nc.gpsimd.to_reg(n)
            return nidx_regs[n]

        for step in range(STEP):
            # ------------------------------------------------ projection
            # emitted lazily per src-subtable so gathers overlap projection
            eng_flip = 0
            PB = 8
            proj_done = [False] * N_CORES

            def emit_proj_r(r, step=step):
                nonlocal eng_flip
                for t0 in range(0, NB, PB):
                    bts = list(range(t0, min(t0 + PB, NB)))
                    o = 128 * t0
                    w = min(128 * PB, Nl - o)
                    g0 = r * Nl + o
                    xt = pools["xt"].tile([D, 128 * PB], BF16, tag="projlhs")
                    ldeng = getattr(nc, LOAD_ENG)
                    if step == 0:
                        ldeng.dma_start(xt[:, :w], xT_in.ap()[:, g0:g0 + w])
                    else:
                        ldeng.dma_start(
                            xt[:, :w], xT_ag.ap()[D * r:D * (r + 1), o:o + w])
                    tb4 = pools["tbl"].tile([128, PB, TW], BF16, tag="tbl")
                    q = (g0 // st_rows) if split_tbl else 0
                    tq = tables[q]
                    gq = g0 - q * st_rows if split_tbl else g0
                    nfull = 0
                    part = None
                    for j, t in enumerate(bts):
                        wj = min(128, Nl - 128 * t)
                        scr = pC.tile([128, 512], F32, tag="scr")
                        nc.tensor.matmul(scr[:wj, 0:TW],
                                         xt[:, 128 * j:128 * j + wj],
                                         waug_t[:], start=True, stop=True)
                        use_dve = (eng_flip % 2 == 0) if COPY_ENG == "alt" \
                            else (COPY_ENG == "dve")
                        if use_dve:
                            nc.vector.tensor_copy(tb4[:wj, j, :],
                                                  scr[:wj, 0:TW])
                        else:
                            nc.scalar.activation(tb4[:wj, j, :],
                                                 scr[:wj, 0:TW], AF.Copy)
                        eng_flip += 1
                        if wj == 128:
                            nfull += 1
                        else:
                            part = (j, wj)
                    if nfull:
                        dst = bass.AP(tq.ap().tensor, gq * TW,
                                      [[TW, 128], [128 * TW, nfull], [1, TW]])
                        nc.sync.dma_start(dst, tb4[:, :nfull, :])
                    if part is not None:
                        j, wj = part
                        gp = gq + 128 * j
                        nc.sync.dma_start(tq.ap()[gp:gp + wj, :],
                                          tb4[:wj, j, :])

            CPQ_T = st_rows // Nl if split_tbl else N_CORES
            LAZY = bool(int(os.environ.get("GAT_LAZYPROJ", "0")))
            if not LAZY:
                for r in range(N_CORES):
                    proj_done[r] = True
                    emit_proj_r(r)

            def ensure_proj(stq):
                if not LAZY:
                    return
                rs = range(CPQ_T * stq, CPQ_T * (stq + 1)) if split_tbl \
                    else range(N_CORES)
                for r in rs:
                    if not proj_done[r]:
                        proj_done[r] = True
                        emit_proj_r(r)

            # ------------------------------------------------ gather + attn
            xt_src = xTl_in if step == 0 else xT_sh
            call_i = 0
            group_i = 0
            for isb in range(NSB):
                blocks = list(range(isb * SB, min((isb + 1) * SB, NB)))
                nb = len(blocks)
                b0 = blocks[0]
                acc = pA.tile([128, SB, 512], F32, tag="acc")
                # x4p = (1-alpha)*x + alpha*lamda*x0 — precombined for step>0
                x4p = pools["blk"].tile([128, SB, D], F32, tag="x4p")
                ldeng = getattr(nc, LOAD_ENG)
                if step == 0:
                    xc4 = pools["blk"].tile([128, SB, 2, D], F32, tag="xc4")
                    ldeng.dma_start(xc4[:, :nb, :, :],
                                    xc_in.ap()[:, b0:b0 + nb, :, :])
                    nc.vector.scalar_tensor_tensor(
                        x4p[:, :nb, :], xc4[:, :nb, 0, :], scal_t[:, 0:1],
                        xc4[:, :nb, 1, :], op0=ALU.mult, op1=ALU.add)
                else:
                    ldeng.dma_start(x4p[:, :nb, :],
                                    xp_mid.ap()[:, b0:b0 + nb, :])
                # er for the superblock: one [D, SB*128] load + per-block matmul
                wsb = min(SB * 128, NBP - 128 * b0)
                xtb4 = pools["xt"].tile([D, SB * 128], BF16, tag="erlhs")
                ldeng.dma_start(
                    xtb4[:, :wsb], xt_src.ap()[:, 128 * b0:128 * b0 + wsb])
                es4 = pools["sm"].tile([128, SB, H], BF16, tag="er4")
                for j, b in enumerate(blocks):
                    nc.tensor.matmul(acc[:, j, 264:264 + H],
                                     xtb4[:, 128 * j:128 * (j + 1)], wr_t[:],
                                     start=True, stop=True)
                    nc.scalar.activation(es4[:, j, :], acc[:, j, 264:264 + H],
                                         AF.Copy)

                # walk this superblock's calls/groups/chunks
                while call_i < len(meta["calls"]):
                    st, lo, nch = meta["calls"][call_i]
                    if lo >= len(meta["chunk_meta"]) or \
                       meta["chunk_meta"][lo][0] != isb:
                        break
                    ensure_proj(st)
                    n = nch * 128
                    rows = pools["rows"].tile([128, MAX_CALL, TW], BF16,
                                              tag="rows")
                    icol = meta["call_cols"][call_i]
                    rows_ap = _ap(rows[:], [TW, nch], [1, TW])
                    if split_tbl:
                        tbl_ap = tables[st].ap()[:, :]
                    else:
                        tbl_ap = tables[0].ap()[st * st_rows:
                                                min((st + 1) * st_rows, N), :]
                    if not SKIP_GATHER:
                        nc.gpsimd.dma_gather(
                            rows_ap, tbl_ap, idx_t[:, icol:icol + n // 16],
                            num_idxs=n, num_idxs_reg=nidx_reg(n), elem_size=TW,
                            single_packet=bool(int(os.environ.get(
                                "GAT_SP1", "0"))),
                            queue_num=call_i % NSWQ)
                    call_i += 1

                    while group_i < len(meta["groups"]):
                        gst, glo_call, g, gs = meta["groups"][group_i]
                        if glo_call != lo:
                            break
                        group_i += 1
                        cc0 = g - lo   # chunk offset within call
                        # one-hot dst mask [128e, gs, 128n] — 2x TT form
                        # (doff2 pair-duplication keeps last dim unit-stride)
                        m8 = pools["mask"].tile([128, GS, 128], BF16, tag="m8")
                        nc.vector.tensor_tensor(
                            _ap(m8[:], [128, gs], [2, 64], [1, 2]),
                            _ap(iota_t[:], [0, gs], [2, 64], [1, 2]),
                            _ap(doff2_t[:, 2 * g:2 * (g + gs)],
                                [2, gs], [0, 64], [1, 2]),
                            op=ALU.is_equal)
                        mT8 = pools["mask"].tile([128, GS, 128], BF16, tag="mT8")
                        if MT8_MODE == "dve":
                            # one is_equal (pre-arranged drep layout) + one
                            # 32x32-block stream transpose per group
                            mt8 = pools["mask"].tile([128, GS, 128], BF16,
                                                     tag="mt8")
                            nc.vector.tensor_tensor(
                                _ap(mt8[:], [128, gs], [1, 128]),
                                _ap(drep_t[:, 4 * g:4 * (g + gs)],
                                    [4, gs], [1, 4], [0, 32]),
                                _ap(itld_t[:], [0, gs], [0, 4], [1, 32]),
                                op=ALU.is_equal)
                            nc.vector.transpose(
                                _ap(mT8[:], [1, gs * 128]),
                                _ap(mt8[:], [1, gs * 128]))
                        else:
                            # per-chunk PE transposes into PSUM (8-chunk
                            # batches), then batched copies (alt DVE/Act)
                            for h0 in range(0, gs, 8):
                                hn = min(8, gs - h0)
                                ms = pB.tile([128, 512], F32, tag="scr8")
                                for k in range(hn):
                                    nc.tensor.transpose(
                                        ms[:, 64 * k:64 * (k + 1)].bitcast(BF16),
                                        m8[:, h0 + k, :], ident_t[:])
                                use_dve = ((group_i + h0) % 2 == 0) \
                                    if COPY_ENG == "alt" \
                                    else (COPY_ENG == "dve")
                                if use_dve:
                                    nc.vector.tensor_copy(
                                        _ap(mT8[:, h0:h0 + hn, :],
                                            [1, hn * 128]),
                                        ms[:, 0:64 * hn].bitcast(BF16))
                                else:
                                    nc.scalar.activation(
                                        _ap(mT8[:, h0:h0 + hn, :],
                                            [1, hn * 128]),
                                        ms[:, 0:64 * hn].bitcast(BF16),
                                        AF.Copy)
                        er8 = pB.tile([128, 512], F32, tag="scr8")
                        for k in range(gs):
                            ci = g + k
                            _, _, b = meta["chunk_meta"][ci]
                            j = b - b0
                            nc.tensor.matmul(er8[:, H * k:H * (k + 1)],
                                             mT8[:, k, :], es4[:, j, :],
                                             start=True, stop=True)
                        # t8 = er[dst] + el[src] (el = first H cols of rows)
                        t8 = pools["sm"].tile([128, GS * H], BF16, tag="t8")
                        nc.vector.tensor_tensor(
                            _ap(t8[:], [H, gs], [1, H]),
                            _ap(er8[:], [H, gs], [1, H]),
                            _ap(rows[:, cc0:cc0 + gs, 0:H], [TW, gs], [1, H]),
                            op=ALU.add)
                        lr8 = pools["sm"].tile([128, GS * H], BF16, tag="lr8")
                        nc.vector.scalar_tensor_tensor(
                            lr8[:, :gs * H], t8[:, :gs * H], scal_t[:, 3:4],
                            t8[:, :gs * H], op0=ALU.mult, op1=ALU.max)
                        rhs8 = pools["rhs"].tile([128, GS, RW], BF16, tag="rhs8")
                        nc.scalar.activation(
                            _ap(rhs8[:], [RW, gs], [1, H]),
                            _ap(lr8[:], [H, gs], [1, H]), AF.Exp)
                        # msg = g_row * exp — 2x TT form (head-minor layout
                        # keeps the exp broadcast's last dim unit-stride)
                        nc.vector.tensor_tensor(
                            _ap(rhs8[:, :, H:RW], [RW, gs], [H, D], [1, H]),
                            _ap(rows[:, cc0:cc0 + gs, :],
                                [TW, gs], [H, D], [1, H]),
                            _ap(rhs8[:], [RW, gs], [0, D], [1, H]),
                            op=ALU.mult)
                        for k in range(gs):
                            ci = g + k
                            _, _, b = meta["chunk_meta"][ci]
                            j = b - b0
                            nc.tensor.matmul(
                                acc[:, j, 0:RW], m8[:, k, :], rhs8[:, k, :],
                                start=(meta["first"][(isb, b)] == ci),
                                stop=(meta["last"][(isb, b)] == ci),
                                skip_group_check=True)

                # ---- superblock epilogue (batched over blocks)
                smax = pools["sm"].tile([128, SB * H], F32, tag="smax")
                nc.vector.tensor_scalar(
                    _ap(smax[:], [H, nb], [1, H]),
                    _ap(acc[:], [512, nb], [1, H]),
                    1e-30, None, op0=ALU.max)
                srec = pools["sm"].tile([128, SB * H], F32, tag="srec")
                nc.vector.reciprocal(srec[:, :nb * H], smax[:, :nb * H])
                srec2 = pools["sm"].tile([128, SB * H], F32, tag="srec2")
                nc.vector.tensor_scalar(
                    srec2[:, :nb * H], srec[:, :nb * H], scal_t[:, 1:2], None,
                    op0=ALU.mult)
                # normalized messages, bf16, head-minor [n, (d,h)]
                onorm = pools["big"].tile([128, SB, HD], BF16, tag="onorm")
                nc.vector.tensor_tensor(
                    _ap(onorm[:], [HD, nb], [H, D], [1, H]),
                    _ap(acc[:, :, H:RW], [512, nb], [H, D], [1, H]),
                    _ap(srec2[:], [H, nb], [0, D], [1, H]),
                    op=ALU.mult)
                xn = pools["blk"].tile([128, SB, D], F32, tag="xn")
                for j, b in enumerate(blocks):
                    # un-rotate + head-mean: out = onorm_j @ Mbig  (256->64)
                    scr = pC.tile([128, 512], F32, tag="scr")
                    nc.tensor.transpose(scr[:, 256:320].bitcast(BF16),
                                        onorm[:, j, 0:128], ident_t[:])
                    nc.tensor.transpose(scr[:, 320:384].bitcast(BF16),
                                        onorm[:, j, 128:256], ident_t[:])
                    oTs = pools["big"].tile([128, 2, 128], BF16, tag="oTs")
                    nc.scalar.activation(_ap(oTs[:], [1, 256]),
                                         scr[:, 256:384].bitcast(BF16),
                                         AF.Copy)
                    red = pB.tile([128, 512], F32, tag="scr8")
                    nc.tensor.matmul(red[:, 0:D], oTs[:, 0, :], mbig_t[:, 0, :],
                                     start=True, stop=False)
                    nc.tensor.matmul(red[:, 0:D], oTs[:, 1, :], mbig_t[:, 1, :],
                                     start=False, stop=True)
                    nc.vector.tensor_add(xn[:, j, :], x4p[:, j, :],
                                         red[:, 0:D])
                if step < STEP - 1:
                    if step > 0:
                        xc4 = pools["blk"].tile([128, SB, 2, D], F32,
                                                tag="xc4")
                        nc.sync.dma_start(
                            xc4[:, :nb, 1, :],
                            xc_in.ap()[:, b0:b0 + nb, 1, :])
                    xp = pools["blk"].tile([128, SB, D], F32, tag="xp")
                    nc.vector.scalar_tensor_tensor(
                        xp[:, :nb, :], xn[:, :nb, :], scal_t[:, 0:1],
                        xc4[:, :nb, 1, :], op0=ALU.mult, op1=ALU.add)
                    nc.sync.dma_start(xp_mid.ap()[:, b0:b0 + nb, :],
                                      xp[:, :nb, :])
                    xnb = pools["blk"].tile([128, SB, D], BF16, tag="xnb")
                    nc.vector.tensor_copy(xnb[:, :nb, :], xn[:, :nb, :])
                    xts4 = pools["sm"].tile([D, SB, 128], BF16, tag="xts")
                    for j, b in enumerate(blocks):
                        scr = pC.tile([128, 512], F32, tag="scr")
                        nc.tensor.transpose(scr[:D, 0:64].bitcast(BF16),
                                            xnb[:, j, :], ident_t[:])
                        nc.scalar.activation(xts4[:, j, :],
                                             scr[:D, 0:64].bitcast(BF16),
                                             AF.Copy)
                    nc.sync.dma_start(
                        xT_sh.ap()[:, 128 * b0:128 * (b0 + nb)],
                        xts4[:, :nb, :])
                else:
                    for j, b in enumerate(blocks):
                        w = tails.get(b, 128)
                        nc.sync.dma_start(x_out.ap()[128 * b:128 * b + w, :],
                                          xn[:w, j, :])
            assert call_i == len(meta["calls"]) and \
                group_i == len(meta["groups"])
            for r in range(N_CORES):
                if not proj_done[r]:
                    proj_done[r] = True
                    emit_proj_r(r)

            if step == 0 and STEP > 1 and not SKIP_COLL:
                nc.gpsimd.collective_compute(
                    "AllGather", ALU.bypass,
                    replica_groups=[list(range(N_CORES))],
                    ins=[xT_sh.ap()[:]], outs=[xT_ag.ap()[:]])

    _split_multi_waits(nc)
    lower_extended_insts(nc)
    return nc


# ----------------------------------------------------------------------------
# entry point
# ----------------------------------------------------------------------------

def _fold_weights(W, attn_l, D, H):
    """Per-head change of basis T_h = D_h @ Householder_h with g = T_h h,
    g[0] = attn_l[h]·h exactly. Returns (waug head-minor [D, H*D],
    Mbig [H*D, D] un-rotation, head-minor rows)."""
    W3 = W.reshape(D, H, D).astype(np.float64)
    waug_hm = np.empty((D, H, D))   # [k, h, d] -> col (d*H + h)
    mbig = np.empty((H, D, D))
    for h in range(H):
        a = attn_l[h].astype(np.float64)
        norm = np.linalg.norm(a)
        v = a / norm
        sign = 1.0 if v[0] >= 0 else -1.0
        u = v.copy()
        u[0] += sign
        u /= np.linalg.norm(u)
        House = np.eye(D) - 2.0 * np.outer(u, u)   # maps v -> -sign*e0
        T = House.copy()
        T[0, :] *= -sign * norm                    # D_h @ House: g[0] = a·h
        waug_hm[:, h, :] = W3[:, h, :] @ T.T
        Minv = House.copy()
        Minv[0, :] *= 1.0 / (-sign * norm)         # Mbig = Dinv @ House:
        mbig[h] = Minv                             # out[n,j] = Σ_k g[n,k]·Mbig[k,j]
    # head-minor interleave: waug[:, d*H + h]
    waug = waug_hm.transpose(0, 2, 1).reshape(D, H * D)
    mbig_hm = mbig.transpose(1, 0, 2).reshape(H * D, D)  # row (d*H+h) -> [D]
    return waug, mbig_hm


def kernel(x, x0, src, dst, W, attn_l, attn_r, alpha, lamda, **kw):
    global _last_results
    x = np.asarray(x, np.float32)
    x0 = np.asarray(x0, np.float32)
    src = np.asarray(src)
    dst = np.asarray(dst)
    W = np.asarray(W, np.float32)
    attn_l = np.asarray(attn_l, np.float32)
    attn_r = np.asarray(attn_r, np.float32)
    alpha_f = float(np.asarray(alpha))
    lamda_f = float(np.asarray(lamda))

    N, D = x.shape
    H = attn_l.shape[0]
    assert N % N_CORES == 0
    if bool(int(os.environ.get("GAT_BALANCE", "1"))):
        perm = _balance_partition(src, dst, N)
    else:
        perm = np.arange(N, dtype=np.int64)
    inv = np.argsort(perm)
    x = x[inv]
    x0 = x0[inv]
    src = perm[src]
    dst = perm[dst]
    meta = _plan_and_arrays(src, dst, N)
    Nl, NB = meta["Nl"], meta["NB"]
    NBP = NB * 128

    nc = _build(meta, N, D, H)

    # host-side weight prep
    waug_f, mbig_f = _fold_weights(W, attn_l, D, H)
    W3 = W.reshape(D, H, D)
    WR = np.einsum("khd,hd->kh", W3, attn_r)
    waug = _bf(waug_f)
    wr = _bf(WR)
    mbig = _bf(mbig_f.reshape(2, 128, D).transpose(1, 0, 2))  # [128, 2, D]
    iota = _bf(np.tile(np.arange(128, dtype=np.float32)[None, :], (128, 1)))
    itld = _bf((32 * (np.arange(128)[:, None] // 32)
                + np.arange(32)[None, :]).astype(np.float32))
    ident = _bf(np.eye(128, dtype=np.float32))
    scal = np.zeros((128, 4), np.float32)
    scal[:, 0] = 1.0 - alpha_f
    scal[:, 1] = alpha_f / H
    scal[:, 3] = NEG_SLOPE
    c0 = (alpha_f * lamda_f) * x0

    xT = _bf(x.T).copy()                      # [D, N]
    in_maps = []
    for p in range(N_CORES):
        lo = p * Nl
        xl = np.zeros((NBP, D), np.float32)
        xl[:Nl] = x[lo:lo + Nl]
        c0l = np.zeros((NBP, D), np.float32)
        c0l[:Nl] = c0[lo:lo + Nl]
        xc = np.stack([xl.reshape(NB, 128, D).transpose(1, 0, 2),
                       c0l.reshape(NB, 128, D).transpose(1, 0, 2)],
                      axis=2)                 # [128, NB, 2, D]
        idxw = meta["idx_wrapped"][p]
        if os.environ.get("GAT_PROBE", "") == "sortidx":
            # timing-only probe: sort each call's indices (breaks results)
            idxw = idxw.copy()
            for ci, (t, lo, nch) in enumerate(meta["calls"]):
                cc = meta["call_cols"][ci]
                n = nch * 128
                seg = idxw[:, cc:cc + n // 16]
                flat = np.sort(seg.T.reshape(-1))
                idxw[:, cc:cc + n // 16] = flat.reshape(-1, 16).T
        in_maps.append({
            "xT_in": np.ascontiguousarray(xT),
            "xTl_in": np.ascontiguousarray(_bf(xl.T)),
            "xc_in": np.ascontiguousarray(xc),
            "waug_in": waug, "wr_in": wr, "mbig_in": mbig,
            "iota_in": iota, "itld_in": itld, "ident_in": ident,
            "scal_in": scal,
            "idx_in": np.ascontiguousarray(np.tile(idxw, (8, 1))),
            "doff2_in": np.ascontiguousarray(_bf(meta["doff2"][p])),
            "drep_in": np.ascontiguousarray(_bf(meta["dstrep"][p])),
        })

    global _last_nc, _last_in_maps
    _last_nc = nc
    _last_in_maps = in_maps
    if os.environ.get("GAT_EXEC", "") == "sim":
        from concourse.bass_interp import MultiCoreSim
        sim = MultiCoreSim(nc, N_CORES, num_workers=N_CORES)
        for p in range(N_CORES):
            for name, arr in in_maps[p].items():
                sim.cores[p].tensor(name)[:] = arr
        sim.simulate()
        out = np.concatenate([np.asarray(sim.cores[p].tensor("x_out"))
                              for p in range(N_CORES)], axis=0)
        return out[perm].astype(np.float32)
    trace = bool(int(os.environ.get("GAT_TRACE", "0")))
    res = run_bass_kernel_spmd(nc, in_maps, core_ids=list(range(N_CORES)),
                               trace=trace,
                               trace_cores=list(range(N_CORES)) if trace else None,
                               stitch_traces=False)
    _last_results = res
    out = np.concatenate([res.results[p]["x_out"] for p in range(N_CORES)],
                         axis=0)
    return out[perm].astype(np.float32)
